# revision 1
# baseline (speedup 1.0000x reference)
"""Self-contained Trainium2 Bass kernel for a 3-layer dense transformer LM.

Model (fp32 reference): embed -> 3x[LN -> MHA(causal) -> +res -> LN -> FFN(gelu) -> +res]
-> LN -> logits.  B=2, S=1024, D=1024, H=16, HD=64, F=4096, V=32000.

Distribution over 8 NeuronCores (one TRN2 chip), Megatron-SP style with
AllToAll instead of reduce-scatter:
  - Residual stream is sequence-sharded: core c owns 256 tokens, kept
    TRANSPOSED in SBUF as rT [D=8x128 partitions-chunks, 256 tokens].
  - LN computed on local tokens (stats via ones-matmul over partition
    chunks), output all-gathered (bf16) so every core has xT [1024, 2048].
  - Attention tensor-parallel over heads: core c computes heads 2c,2c+1
    for ALL tokens; per-head causal softmax without max subtraction
    (scores are small); denominators come free via a ones-column in v.
  - AllToAll converts head-sharded attention output to token-sharded,
    then each core applies the FULL Wo for its own 256 tokens. Same
    pattern for FFN: W1 column-sharded, AllToAll, full W2 locally.
  - Logits: final LN -> AllGather -> each core computes a 4000-column
    vocab slice for all 2048 tokens; host concatenates.

Compute dtype bf16 (PE full rate), accumulation fp32 in PSUM.
"""

import numpy as np
import ml_dtypes

BF = ml_dtypes.bfloat16

B, S, D, H, L, F, V = 2, 1024, 1024, 16, 3, 4096, 32000
HD = D // H
T = B * S            # 2048 tokens
NC = 8               # cores
TLOC = T // NC       # 256 tokens per core
VS = V // NC         # 4000 vocab cols per core
EPS = 1e-5
DCH = D // 128       # 8 partition chunks of the hidden dim
FCH_LOC = F // NC // 128   # 4 chunks of the local FFN shard
VCH = 32             # vocab m-chunks per core
VMC = VS // VCH      # 125 vocab cols per m-chunk


def _build(n_layers, use_bout, ln_triv, debug=False):
    import concourse.bass as bass
    import concourse.mybir as mybir
    import concourse.tile as tile
    from concourse import bacc

    F32 = mybir.dt.float32
    BF16 = mybir.dt.bfloat16
    AF = mybir.ActivationFunctionType
    OP = mybir.AluOpType

    nc = bacc.Bacc("TRN2", target_bir_lowering=False, debug=False,
                   num_devices=NC)
    RG = [list(range(NC))]

    # ---------------- external parameters (per-core shards) ---------------
    ext = {}
    def inp(name, shape, dt=F32):
        ext[name] = nc.dram_tensor(name, shape, dt, kind="ExternalInput")
        return ext[name]

    embT = inp("embT", [128, DCH, TLOC])
    posT = inp("posT", [128, DCH, TLOC])
    wq = inp("wq", [n_layers, DCH, 128, DCH, 128], BF16)
    wk = inp("wk", [n_layers, DCH, 128, DCH, 128], BF16)
    wv = inp("wv", [n_layers, DCH, 128, DCH, 128], BF16)
    wo = inp("wo", [n_layers, 128, DCH, D], BF16)
    w1 = inp("w1", [n_layers, 32, 128, DCH, 128], BF16)
    w2 = inp("w2", [n_layers, DCH, 128, 32, 128], BF16)
    wout = inp("wout", [VCH, 128, DCH, VMC], BF16)
    bqkv = inp("bqkv", [128, DCH, 3, n_layers])
    boT = inp("boT", [128, DCH, n_layers])
    b1T = inp("b1T", [128, 32, n_layers])
    b2T = inp("b2T", [128, DCH, n_layers])
    boutT = inp("boutT", [VMC, VCH])
    g1T = inp("g1T", [128, DCH, n_layers])
    be1T = inp("be1T", [128, DCH, n_layers])
    g2T = inp("g2T", [128, DCH, n_layers])
    be2T = inp("be2T", [128, DCH, n_layers])
    gfT = inp("gfT", [128, DCH, 1])
    befT = inp("befT", [128, DCH, 1])
    cmask = inp("cmask", [128, 2, TLOC], BF16)
    ident = inp("ident", [128, 128], BF16)

    outT = nc.dram_tensor("outT", [VS, T], F32, kind="ExternalOutput")

    dbg = {}
    def dbg_out(name, shape, dt=F32):
        if debug:
            dbg[name] = nc.dram_tensor(name, shape, dt, kind="ExternalOutput")
        return dbg.get(name)

    # ---------------- internal DRAM (collective bounce buffers) -----------
    HD2 = DCH // 2
    NSITE = 1
    ag_in = [nc.dram_tensor(f"ag{s_}_in", [128, DCH, TLOC], BF16)
             for s_ in range(NSITE)]
    ag_out = [nc.dram_tensor(f"ag{s_}_out", [NC * 128, DCH, TLOC], BF16,
                             addr_space="Shared") for s_ in range(NSITE)]
    warm_in = nc.dram_tensor("warm_in", [128, 16], BF16)
    warm_out = nc.dram_tensor("warm_out", [NC * 128, 16], BF16,
                              addr_space="Shared")
    warm2_in = nc.dram_tensor("warm2_in", [NC, 16, 16], BF16)
    warm2_out = nc.dram_tensor("warm2_out", [NC, 16, 16], BF16)
    a2a1_in, a2a1_out = [], []
    aqkv_in, aqkv_out = [], []
    for i in range(n_layers):
        aqkv_in.append([nc.dram_tensor(f"aqkv{i}{j}_in",
                                       [NC, 128 * (2 - j), TLOC], BF16)
                        for j in range(2)])
        aqkv_out.append([nc.dram_tensor(f"aqkv{i}{j}_out",
                                        [NC, 128 * (2 - j), TLOC], BF16)
                         for j in range(2)])
        a2a1_in.append([nc.dram_tensor(f"a2a1{i}{h}_in", [NC, 64, TLOC], BF16)
                        for h in range(2)])
        a2a1_out.append([nc.dram_tensor(f"a2a1{i}{h}_out", [NC, 64, TLOC],
                                        BF16) for h in range(2)])

    with tile.TileContext(nc) as tc:
        with tc.tile_pool(name="const", bufs=1) as konst, \
             tc.tile_pool(name="resid", bufs=1) as resid, \
             tc.tile_pool(name="acts", bufs=1) as acts, \
             tc.tile_pool(name="wpool", bufs=2) as wpool, \
             tc.tile_pool(name="wop", bufs=3) as wop, \
             tc.tile_pool(name="wob", bufs=4) as wob, \
             tc.tile_pool(name="lnp", bufs=1) as lnp, \
             tc.tile_pool(name="otp", bufs=2) as otp, \
             tc.tile_pool(name="att", bufs=1) as att, \
             tc.tile_pool(name="tmp", bufs=3) as tmp, \
             tc.tile_pool(name="expp", bufs=6) as expp, \
             tc.tile_pool(name="ps", bufs=2, space="PSUM") as ps, \
             tc.tile_pool(name="ps3", bufs=3, space="PSUM") as ps3, \
             tc.tile_pool(name="ps1", bufs=1, space="PSUM") as ps1:

            # ---- constants resident in SBUF ----
            msk = konst.tile([128, 2, TLOC], BF16)
            nc.scalar.dma_start(out=msk[:], in_=cmask[:, :, :])
            idn = konst.tile([128, 128], BF16)
            nc.scalar.dma_start(out=idn[:], in_=ident[:, :])
            ones_bf = konst.tile([128, 1], BF16)
            nc.vector.memset(ones_bf[:], 1.0)
            eps_sb = konst.tile([1, 1], F32)
            nc.vector.memset(eps_sb[:], EPS)
            warm = konst.tile([1, 1], F32)
            nc.vector.memset(warm[:], 1.0)
            bqkv_sb = konst.tile([128, DCH, 3, n_layers], F32)
            nc.scalar.dma_start(out=bqkv_sb[:], in_=bqkv[:, :, :])
            bo_sb = konst.tile([128, DCH, n_layers], F32)
            nc.scalar.dma_start(out=bo_sb[:], in_=boT[:, :, :])
            b1_sb = konst.tile([128, 32, n_layers], F32)
            nc.scalar.dma_start(out=b1_sb[:], in_=b1T[:, :, :])
            b2_sb = konst.tile([128, DCH, n_layers], F32)
            nc.scalar.dma_start(out=b2_sb[:], in_=b2T[:, :, :])
            lng = {}
            for nm, t_ in (("g1", g1T), ("be1", be1T), ("g2", g2T),
                           ("be2", be2T), ("gf", gfT), ("bef", befT)):
                nl = 1 if nm in ("gf", "bef") else n_layers
                lnt = konst.tile([128, DCH, nl], F32, tag=f"ln_{nm}",
                                 name=f"ln_{nm}")
                nc.scalar.dma_start(out=lnt[:], in_=t_[:, :, :])
                lng[nm] = lnt

            def prewarm(func):
                nc.scalar.activation(warm[:], warm[:], func)

            # ---- warm up ncfw/collective path early ----
            wtile = konst.tile([128, 16], BF16)
            nc.vector.memset(wtile[:], 0.0)
            nc.sync.dma_start(out=warm_in[:, :], in_=wtile[:])
            nc.gpsimd.collective_compute(
                "AllGather", OP.bypass, replica_groups=RG,
                ins=[warm_in.ap().opt()], outs=[warm_out.ap().opt()])


            # ---- residual init: rT = embT + posT ----
            rT = resid.tile([128, DCH, TLOC], F32)
            pt = acts.tile([128, DCH, TLOC], F32, tag="scratch8k")
            nc.sync.dma_start(out=rT[:], in_=embT[:, :, :])
            nc.sync.dma_start(out=pt[:], in_=posT[:, :, :])
            nc.vector.tensor_add(rT[:], rT[:], pt[:])
            if debug:
                o = dbg_out("dbg_rT0", [128, DCH, TLOC])
                nc.sync.dma_start(out=o[:, :, :], in_=rT[:])

            # ------------- layernorm machinery (interleaved stats) --------
            class LN:
                def begin(self):
                    self.xs = lnp.tile([128, 2, DCH, TLOC], BF16,
                                       tag="ln_xs", name="ln_xs")
                    self.st = ps1.tile([1, 2, TLOC], F32, tag="pstat",
                                       name="ln_st")

                def chunk(self, c):
                    nc.vector.tensor_copy(self.xs[:, 0, c, :], rT[:, c, :])
                    nc.scalar.square(self.xs[:, 1, c, :], rT[:, c, :])
                    nc.tensor.matmul(self.st[:], ones_bf[:],
                                     self.xs[:, :, c, :],
                                     start=(c == 0), stop=(c == DCH - 1))

                def finish(self, gname, bname, li, triv, site):
                    st = self.st
                    m1 = tmp.tile([1, TLOC], F32, tag="ln_m1", name="m1")
                    nc.vector.tensor_scalar_mul(m1[:], st[:, 0, :], 1.0 / D)
                    t2 = tmp.tile([1, TLOC], F32, tag="ln_t2", name="t2")
                    nc.vector.tensor_mul(t2[:], m1[:], m1[:])
                    v_ = tmp.tile([1, TLOC], F32, tag="ln_v", name="v_")
                    nc.vector.scalar_tensor_tensor(
                        v_[:], st[:, 1, :], 1.0 / D, t2[:],
                        op0=OP.mult, op1=OP.subtract)
                    nc.scalar.activation(t2[:], v_[:], AF.Sqrt,
                                         bias=eps_sb[:])
                    rstd = v_
                    nc.vector.reciprocal_approx_fast(out=rstd[:], in_=t2[:])
                    off = t2
                    nc.vector.scalar_tensor_tensor(
                        off[:], m1[:], -1.0, rstd[:],
                        op0=OP.mult, op1=OP.mult)
                    rstd_b = lnp.tile([128, TLOC], F32, tag="ln_rb",
                                      name="rb")
                    nc.gpsimd.partition_broadcast(rstd_b[:], rstd[:],
                                                  channels=128)
                    off_b = lnp.tile([128, TLOC], F32, tag="ln_ob", name="ob")
                    nc.gpsimd.partition_broadcast(off_b[:], off[:],
                                                  channels=128)
                    xo = lnp.tile([128, DCH, TLOC], BF16, tag="ln_out",
                                  name="xo")
                    rb3 = rstd_b[:, None, :].to_broadcast([128, HD2, TLOC])
                    ob3 = off_b[:, None, :].to_broadcast([128, HD2, TLOC])
                    self.xo = xo
                    for hh in range(2):
                        cs = slice(HD2 * hh, HD2 * (hh + 1))
                        if triv:
                            t_ = acts.tile([128, HD2, TLOC], F32,
                                           tag="scratch8k", name="lt")
                            nc.vector.tensor_mul(t_[:], rT[:, cs, :], rb3)
                            nc.vector.tensor_add(xo[:, cs, :], t_[:], ob3)
                        else:
                            for c in range(HD2 * hh, HD2 * (hh + 1)):
                                tc_ = lnp.tile([128, TLOC], F32,
                                               tag="ln_tc", name="ltc")
                                nc.vector.tensor_mul(tc_[:], rT[:, c, :],
                                                     rstd_b[:])
                                nc.vector.tensor_add(tc_[:], tc_[:],
                                                     off_b[:])
                                nc.vector.tensor_scalar(
                                    xo[:, c, :], tc_[:],
                                    lng[gname][:, c, li:li + 1],
                                    lng[bname][:, c, li:li + 1],
                                    op0=OP.mult, op1=OP.add)
                        if site is not None:
                            nc.sync.dma_start(out=ag_in[site][:, cs, :],
                                              in_=xo[:, cs, :])
                    if site is not None:
                        nc.gpsimd.collective_compute(
                            "AllGather", OP.bypass, replica_groups=RG,
                            ins=[ag_in[site].ap().opt()],
                            outs=[ag_out[site].ap().opt()])

            ln = LN()

            def ag_load(site):
                """Gathered output -> xtf [128, NC, DCH, TLOC], chunk-pair
                DMAs so consumers start on the first pair."""
                xtf = acts.tile([128, NC, DCH, TLOC], BF16, tag="bigact",
                                name="xtf")
                v = ag_out[site].ap().rearrange("(r p) c t -> p r c t", p=128)
                for q in range(4):
                    nc.sync.dma_start(
                        out=xtf[:, :, 2 * q:2 * q + 2, :],
                        in_=v[:, :, 2 * q:2 * q + 2, :])
                return xtf

            # LN1 of layer 0 (stats over freshly built rT)
            ln.begin()
            for c in range(DCH):
                ln.chunk(c)
            ln.finish("g1", "be1", 0, ln_triv[0], None)
            prewarm(AF.Exp)

            # =================== layers ===================
            for li in range(n_layers):
                # ---- QKV computed on LOCAL tokens with full weights;
                #      one AllToAll per projection, pipelined ----
                xo = ln.xo
                qT = acts.tile([128, T], BF16, tag="qT")
                kT = acts.tile([128, T], BF16, tag="kT")
                vT = acts.tile([128, T], BF16, tag="vT")
                qkl = att.tile([128, 2, DCH, TLOC], BF16, tag="qkvloc",
                               name="qkl")
                for j, wext in enumerate((wq, wk, wv)):
                    for cc in range(DCH):
                        wt = wop.tile([128, DCH, 128], BF16, tag="wom",
                                      name=f"wt{j}{cc}")
                        nc.scalar.dma_start(out=wt[:],
                                            in_=wext[li, cc, :, :, :])
                        pp = ps3.tile([128, TLOC], F32, tag="psc",
                                      name=f"qp{j}{cc}")
                        for d in range(DCH):
                            nc.tensor.matmul(pp[:], wt[:, d, :], xo[:, d, :],
                                             start=(d == 0),
                                             stop=(d == DCH - 1))
                        dstl = qkl[:, j, cc, :] if j < 2 else qkl[:, 0, cc, :]
                        if j == 2:
                            dstl = qkl[:, 1, cc, :]
                        dstl = qkl[:, min(j, 1), cc, :]
                        nc.vector.tensor_scalar(
                            dstl, pp[:],
                            bqkv_sb[:, cc, j, li:li + 1], None, op0=OP.add)
                    if j == 1:
                        # merged q+k AllToAll
                        for jj in range(2):
                            nc.sync.dma_start(
                                out=aqkv_in[li][0].ap().rearrange(
                                    "r (j p) t -> p j r t", p=128)[:, jj],
                                in_=qkl[:, jj, :, :])
                        nc.gpsimd.collective_compute(
                            "AllToAll", OP.bypass, replica_groups=RG,
                            ins=[aqkv_in[li][0].ap().opt()],
                            outs=[aqkv_out[li][0].ap().opt()])
                        vv = aqkv_out[li][0].ap().rearrange(
                            "r (j p) t -> p j r t", p=128)
                        nc.sync.dma_start(
                            out=qT[:].rearrange("p (r t) -> p r t", r=NC),
                            in_=vv[:, 0, :, :])
                        nc.sync.dma_start(
                            out=kT[:].rearrange("p (r t) -> p r t", r=NC),
                            in_=vv[:, 1, :, :])
                    elif j == 2:
                        nc.sync.dma_start(
                            out=aqkv_in[li][1].ap().rearrange(
                                "r p t -> p r t"),
                            in_=qkl[:, 1, :, :])
                        nc.gpsimd.collective_compute(
                            "AllToAll", OP.bypass, replica_groups=RG,
                            ins=[aqkv_in[li][1].ap().opt()],
                            outs=[aqkv_out[li][1].ap().opt()])
                        nc.sync.dma_start(
                            out=vT[:].rearrange("p (r t) -> p r t", r=NC),
                            in_=aqkv_out[li][1].ap().rearrange(
                                "r p t -> p r t"))
                v_aug = acts.tile([128, 16, 130], BF16, tag="vaug")
                nc.vector.memset(v_aug[:, :, 64:65], 1.0)
                nc.vector.memset(v_aug[:, :, 129:130], 1.0)
                for tt in range(16):
                    tp = ps1.tile([128, 128], BF16, tag="ptr", name="tp")
                    nc.tensor.transpose(tp[:], vT[:, 128 * tt:128 * (tt + 1)],
                                        idn[:])
                    nc.vector.tensor_copy(v_aug[:, tt, 0:64], tp[:, 0:64])
                    nc.vector.tensor_copy(v_aug[:, tt, 65:129], tp[:, 64:128])
                if debug and li == 0:
                    for nm, t_ in (("dbg_qT", qT), ("dbg_kT", kT),
                                   ("dbg_vaug", v_aug)):
                        o = dbg_out(nm, list(t_.shape), BF16)
                        nc.sync.dma_start(out=o.ap().opt(), in_=t_[:])

                # ---- attention (head-outer; per-head AllToAll) ----
                hoT = acts.tile([64, 2, T], BF16, tag="hoT")
                for h in range(2):
                    for b in range(B):
                        hb = 64 * h
                        dnc = att.tile([1, 1024], F32, tag="dnc", name="dnc")
                        for qs in range(4):
                            av = ps1.tile([65, TLOC], F32, tag="pav",
                                         name="av")
                            nkc = 2 * qs + 2
                            for kc in range(nkc):
                                sc = ps3.tile([128, TLOC], F32, tag="psc",
                                             name="sc")
                                nc.tensor.matmul(
                                    sc[:],
                                    kT[hb:hb + 64,
                                       1024 * b + 128 * kc:
                                       1024 * b + 128 * (kc + 1)],
                                    qT[hb:hb + 64,
                                       1024 * b + 256 * qs:
                                       1024 * b + 256 * (qs + 1)],
                                    start=True, stop=True)
                                et_ = expp.tile([128, TLOC], BF16, tag="exp",
                                                name="et")
                                nc.scalar.activation(et_[:], sc[:], AF.Exp,
                                                     scale=float(HD) ** -0.5)
                                if kc >= 2 * qs:
                                    mi = 0 if kc == 2 * qs else 1
                                    nc.vector.tensor_mul(et_[:], et_[:],
                                                         msk[:, mi, :])
                                nc.tensor.matmul(
                                    av[:],
                                    v_aug[:, 8 * b + kc, 65 * h:65 * (h + 1)],
                                    et_[:],
                                    start=(kc == 0), stop=(kc == nkc - 1))
                            nc.vector.tensor_copy(
                                dnc[:, 256 * qs:256 * (qs + 1)],
                                av[64:65, :])
                            nc.vector.tensor_copy(
                                hoT[:, h, 1024 * b + 256 * qs:
                                    1024 * b + 256 * (qs + 1)],
                                av[0:64, :])
                        rc2 = att.tile([1, 1024], F32, tag="rc2", name="rc2")
                        nc.vector.reciprocal_approx_fast(out=rc2[:],
                                                         in_=dnc[:])
                        rcb = att.tile([64, 1024], F32, tag="rcb", name="rcb")
                        nc.gpsimd.partition_broadcast(rcb[:], rc2[:],
                                                      channels=64)
                        nc.vector.tensor_mul(
                            hoT[:, h, 1024 * b:1024 * (b + 1)],
                            hoT[:, h, 1024 * b:1024 * (b + 1)], rcb[:])
                    nc.sync.dma_start(
                        out=a2a1_in[li][h].ap().rearrange("r p t -> p r t"),
                        in_=hoT[:, h, :].rearrange("p (r t) -> p r t", r=NC))
                    nc.gpsimd.collective_compute(
                        "AllToAll", OP.bypass, replica_groups=RG,
                        ins=[a2a1_in[li][h].ap().opt()],
                        outs=[a2a1_out[li][h].ap().opt()])
                prewarm(AF.Sqrt)
                if debug and li == 0:
                    o = dbg_out("dbg_hoT", [64, 2, T], BF16)
                    nc.sync.dma_start(out=o[:, :, :], in_=hoT[:])

                # ---- Wo (head-major rearranged rows) + LN2 stats hooks ----
                acs = []
                for h in range(2):
                    ach = acts.tile([128, FCH_LOC, TLOC], BF16,
                                    tag=f"hoac{h}", name=f"ac{h}")
                    nc.sync.dma_start(
                        out=ach[:],
                        in_=a2a1_out[li][h].ap().rearrange(
                            "(j k) p t -> (k p) j t", k=2))
                    acs.append(ach)
                ln.begin()
                wo_sbs = []
                for m in range(DCH):
                    wo_sb = wob.tile([128, DCH, 128], BF16, tag=f"wom{m % 2}",
                                     name=f"wo_sb{m}")
                    nc.scalar.dma_start(
                        out=wo_sb[:], in_=wo[li, :, :, 128 * m:128 * (m + 1)])
                    wo_sbs.append(wo_sb)
                    pp = ps3.tile([128, TLOC], F32, tag="psc", name="wop_ps")
                    for kc in range(4):
                        nc.tensor.matmul(pp[:], wo_sb[:, kc, :],
                                         acs[0][:, kc, :],
                                         start=(kc == 0), stop=(kc == 3))
                    nc.vector.scalar_tensor_tensor(
                        rT[:, m, :], pp[:], bo_sb[:, m, li:li + 1],
                        rT[:, m, :], op0=OP.add, op1=OP.add)
                for m in range(DCH):
                    pp = ps3.tile([128, TLOC], F32, tag="psc", name="wop_ps2")
                    for kc in range(4):
                        nc.tensor.matmul(pp[:], wo_sbs[m][:, 4 + kc, :],
                                         acs[1][:, kc, :],
                                         start=(kc == 0), stop=(kc == 3))
                    nc.vector.tensor_add(rT[:, m, :], rT[:, m, :], pp[:])
                    ln.chunk(m)
                if debug and li == 0:
                    o = dbg_out("dbg_rT1", [128, DCH, TLOC])
                    nc.sync.dma_start(out=o[:, :, :], in_=rT[:])
                ln.finish("g2", "be2", li, ln_triv[2 * li + 1], None)
                prewarm(AF.Gelu)

                # ---- FFN fully local: W1 full + gelu -> W2 full ----
                xo2 = ln.xo
                h1 = acts.tile([128, 32, TLOC], BF16, tag="bigact",
                               name="h1")
                for cc in range(32):
                    wt1 = wop.tile([128, DCH, 128], BF16, tag="wom",
                                   name=f"wt1{cc}")
                    nc.scalar.dma_start(out=wt1[:], in_=w1[li, cc, :, :, :])
                    pp = ps3.tile([128, TLOC], F32, tag="psc",
                                  name=f"w1p{cc}")
                    for d in range(DCH):
                        nc.tensor.matmul(pp[:], wt1[:, d, :], xo2[:, d, :],
                                         start=(d == 0), stop=(d == DCH - 1))
                    nc.scalar.activation(h1[:, cc, :], pp[:], AF.Gelu,
                                         bias=b1_sb[:, cc, li:li + 1])
                if debug and li == 0:
                    o = dbg_out("dbg_h1T", [128, 32, TLOC], BF16)
                    nc.sync.dma_start(out=o[:, :, :], in_=h1[:])

                # ---- full W2 + residual + next-LN stats hooks ----
                ln.begin()
                for m in range(DCH):
                    w2_sb = wop.tile([128, 32, 128], BF16, tag="w2",
                                     name="w2_sb")
                    nc.scalar.dma_start(out=w2_sb[:], in_=w2[li, m, :, :, :])
                    pp = ps3.tile([128, TLOC], F32, tag="psc", name="w2_ps")
                    for kc in range(32):
                        nc.tensor.matmul(pp[:], w2_sb[:, kc, :],
                                         h1[:, kc, :],
                                         start=(kc == 0), stop=(kc == 31))
                    nc.vector.scalar_tensor_tensor(
                        rT[:, m, :], pp[:], b2_sb[:, m, li:li + 1],
                        rT[:, m, :], op0=OP.add, op1=OP.add)
                    ln.chunk(m)
                if debug and li == 0:
                    o = dbg_out("dbg_rT2", [128, DCH, TLOC])
                    nc.sync.dma_start(out=o[:, :, :], in_=rT[:])
                if li + 1 < n_layers:
                    ln.finish("g1", "be1", li + 1, ln_triv[2 * (li + 1)],
                              None)
                    prewarm(AF.Exp)
                else:
                    ln.finish("gf", "bef", 0, ln_triv[2 * n_layers], 0)

            # =================== logits ===================
            xtff = ag_load(0)
            bout_sb = None
            if use_bout:
                bout_sb = konst.tile([VMC, VCH], F32)
                nc.scalar.dma_start(out=bout_sb[:], in_=boutT[:, :])
            for m in range(VCH):
                wo_t = wop.tile([128, DCH, VMC], BF16, tag="wout",
                                name="wo_t")
                nc.scalar.dma_start(out=wo_t[:], in_=wout[m, :, :, :])
                ot = otp.tile([VMC, T], F32, tag="ot", name="ot")
                pps = [ps.tile([VMC, 512], F32, tag="p512",
                               name=f"plga{k}") for k in range(2)] + \
                      [ps3.tile([VMC, 512], F32, tag="psc",
                                name=f"plgb{k}") for k in range(2)]
                for d in range(DCH):
                    for n in range(4):
                        nc.tensor.matmul(pps[n][:], wo_t[:, d, :],
                                         xtff[:, 2 * n:2 * n + 2, d, :],
                                         start=(d == 0),
                                         stop=(d == DCH - 1))
                for n in range(4):
                    if use_bout:
                        nc.vector.tensor_scalar(
                            ot[:, 512 * n:512 * (n + 1)], pps[n][:],
                            bout_sb[:, m:m + 1], None, op0=OP.add)
                    else:
                        nc.scalar.copy(ot[:, 512 * n:512 * (n + 1)],
                                       pps[n][:])
                nc.sync.dma_start(out=outT[VMC * m:VMC * (m + 1), :],
                                  in_=ot[:])

    nc.compile()
    return nc, sorted(dbg.keys())


def _prep_inputs(inputs):
    """Shard + transform full inputs -> list of 8 per-core in_maps."""
    f32 = lambda a: np.ascontiguousarray(np.asarray(a), dtype=np.float32)
    bf16 = lambda a: np.ascontiguousarray(np.asarray(a, dtype=np.float32)).astype(BF)

    x = np.asarray(inputs["x"]).astype(np.int64).reshape(T)
    tok = f32(inputs["tok_emb"])
    pos = f32(inputs["pos_emb"])
    Wq, Wk, Wv, Wo = (f32(inputs[k]) for k in ("Wq", "Wk", "Wv", "Wo"))
    W1, W2, Wout = f32(inputs["W1"]), f32(inputs["W2"]), f32(inputs["Wout"])
    bq, bk, bv = f32(inputs["bq"]), f32(inputs["bk"]), f32(inputs["bv"])
    bo, b1, b2 = f32(inputs["bo"]), f32(inputs["b1"]), f32(inputs["b2"])
    bout = f32(inputs["bout"])
    ln1_g, ln1_b = f32(inputs["ln1_g"]), f32(inputs["ln1_b"])
    ln2_g, ln2_b = f32(inputs["ln2_g"]), f32(inputs["ln2_b"])
    lnf_g, lnf_b = f32(inputs["lnf_g"]), f32(inputs["lnf_b"])

    def chunkT(vec_1d):  # [D] -> [128, DCH] (p, chunk)
        return np.ascontiguousarray(vec_1d.reshape(DCH, 128).T)

    # causal mask tiles: keep if f >= p + 128*r
    p_i = np.arange(128)[:, None]
    f_i = np.arange(TLOC)[None, :]
    cm = np.stack([(f_i >= p_i), (f_i >= p_i + 128)], 1).astype(BF)  # [128,2,256]
    idn = np.eye(128, dtype=BF)

    ln_triv = []
    for g, b in ((ln1_g, ln1_b), (ln2_g, ln2_b)):
        for i in range(L):
            ln_triv.append(bool(np.all(g[i] == 1.0) and np.all(b[i] == 0.0)))
    # interleave per layer: [ln1_0, ln2_0, ln1_1, ln2_1, ...]
    ln_triv = [ln_triv[i] if s == 0 else ln_triv[L + i]
               for i in range(L) for s in range(2)] + \
              [bool(np.all(lnf_g == 1.0) and np.all(lnf_b == 0.0))]
    use_bout = bool(np.any(bout))

    in_maps = []
    for c in range(NC):
        xl = x[TLOC * c:TLOC * (c + 1)]
        embT = np.ascontiguousarray(tok[xl].T).reshape(DCH, 128, TLOC)
        embT = np.ascontiguousarray(embT.transpose(1, 0, 2))  # [128, DCH, 256]
        p0 = (c % 4) * TLOC
        posT = np.ascontiguousarray(pos[p0:p0 + TLOC].T).reshape(DCH, 128, TLOC)
        posT = np.ascontiguousarray(posT.transpose(1, 0, 2))
        hc = slice(128 * c, 128 * (c + 1))
        lay = lambda w: np.ascontiguousarray(
            w.reshape(L, DCH, 128, w.shape[-1]).transpose(0, 2, 1, 3))
        qlay = lambda w: np.ascontiguousarray(
            w.reshape(L, DCH, 128, DCH, 128).transpose(0, 3, 2, 1, 4))
        w2h = np.ascontiguousarray(
            W2.reshape(L, 32, 128, DCH, 128).transpose(0, 3, 2, 1, 4))
        wouth = np.ascontiguousarray(
            Wout[:, VS * c:VS * (c + 1)].reshape(DCH, 128, VCH, VMC)
            .transpose(2, 1, 0, 3))
        m = dict(
            embT=embT, posT=posT,
            wq=bf16(qlay(Wq)), wk=bf16(qlay(Wk)), wv=bf16(qlay(Wv)),
            wo=bf16(lay(Wo.reshape(L, NC, 2, 64, D).transpose(0, 2, 1, 3, 4)
                        .reshape(L, D, D))),
            w1=bf16(np.ascontiguousarray(
                W1.reshape(L, DCH, 128, 32, 128).transpose(0, 3, 2, 1, 4))),
            w2=bf16(w2h), wout=bf16(wouth),
            bqkv=np.ascontiguousarray(
                np.stack([np.stack([chunkT(bb[i]) for i in range(L)], -1)
                          for bb in (bq, bk, bv)], 2)),  # [128,DCH,3,L]
            boT=np.ascontiguousarray(
                np.stack([chunkT(bo[i]) for i in range(L)], -1)),
            b1T=np.ascontiguousarray(np.stack(
                [np.ascontiguousarray(b1[i].reshape(32, 128).T)
                 for i in range(L)], -1)),
            b2T=np.ascontiguousarray(
                np.stack([chunkT(b2[i]) for i in range(L)], -1)),
            boutT=np.ascontiguousarray(
                bout[VS * c:VS * (c + 1)].reshape(VCH, VMC).T),
            g1T=np.ascontiguousarray(
                np.stack([chunkT(ln1_g[i]) for i in range(L)], -1)),
            be1T=np.ascontiguousarray(
                np.stack([chunkT(ln1_b[i]) for i in range(L)], -1)),
            g2T=np.ascontiguousarray(
                np.stack([chunkT(ln2_g[i]) for i in range(L)], -1)),
            be2T=np.ascontiguousarray(
                np.stack([chunkT(ln2_b[i]) for i in range(L)], -1)),
            gfT=chunkT(lnf_g)[:, :, None].copy(),
            befT=chunkT(lnf_b)[:, :, None].copy(),
            cmask=cm, ident=idn,
        )
        in_maps.append(m)
    return in_maps, use_bout, ln_triv


_CACHE = {}


def _get_nc(n_layers, use_bout, ln_triv, debug=False):
    key = (n_layers, use_bout, tuple(ln_triv), debug)
    if key not in _CACHE:
        _CACHE[key] = _build(n_layers, use_bout, ln_triv, debug)
    return _CACHE[key]


def run(inputs, n_layers=L, debug=False, trace=False):
    from concourse import bass_utils
    in_maps, use_bout, ln_triv = _prep_inputs(inputs)
    nc, dbg_names = _get_nc(n_layers, use_bout, ln_triv, debug)
    res = bass_utils.run_bass_kernel_spmd(
        nc, in_maps, core_ids=list(range(NC)), trace=trace)
    return res, dbg_names


def kernel(**inputs):
    res, _ = run(inputs)
    outs = [res.results[c]["outT"] for c in range(NC)]          # [4000, 2048]
    logits = np.concatenate([o.T for o in outs], axis=1)        # [2048, 32000]
    return np.ascontiguousarray(logits.reshape(B, S, V))



# revision 11
# speedup vs baseline: 1.1137x; 1.1137x over previous
"""Self-contained Trainium2 Bass kernel for a 3-layer dense transformer LM.

Model (fp32 reference): embed -> 3x[LN -> MHA(causal) -> +res -> LN -> FFN(gelu) -> +res]
-> LN -> logits.  B=2, S=1024, D=1024, H=16, HD=64, F=4096, V=32000.

Distribution over 8 NeuronCores (one TRN2 chip), Megatron-SP style with
AllToAll instead of reduce-scatter:
  - Residual stream is sequence-sharded: core c owns 256 tokens, kept
    TRANSPOSED in SBUF as rT [D=8x128 partitions-chunks, 256 tokens].
  - LN computed on local tokens (stats via ones-matmul over partition
    chunks).
  - Attention tensor-parallel over heads: core c computes heads 2c,2c+1
    for ALL tokens; per-head causal softmax without max subtraction
    (scores are small); denominators come free via a ones-column in v.
    Softmax is key-block-outer: one exp per key block over the full
    valid query range (few large ACT ops instead of many small ones).
  - AllToAll converts head-sharded attention output to token-sharded,
    then each core applies the FULL Wo for its own 256 tokens. Same
    pattern for FFN: full W1 + full W2 locally (no collective).
  - Logits: final LN -> AllGather -> each core computes a 4000-column
    vocab slice for all 2048 tokens; host concatenates.

Compute dtype bf16 (PE full rate), accumulation fp32 in PSUM, output
logits written bf16.  Weight streaming: 512KB-1MB batched DMAs split
across the two HWDGE rings (sync=qSP for QKV/W1/wout, scalar=qAct for
Wo/W2/outputs); collective staging on the gpsimd SWDGE ring.
"""

import numpy as np
import ml_dtypes

BF = ml_dtypes.bfloat16

B, S, D, H, L, F, V = 2, 1024, 1024, 16, 3, 4096, 32000
HD = D // H
T = B * S            # 2048 tokens
NC = 8               # cores
TLOC = T // NC       # 256 tokens per core
VS = V // NC         # 4000 vocab cols per core
EPS = 1e-5
DCH = D // 128       # 8 partition chunks of the hidden dim
VCH = 32             # vocab m-chunks per core
VMC = VS // VCH      # 125 vocab cols per m-chunk


def _build(n_layers, use_bout, ln_triv, debug=False):
    import concourse.bass as bass
    import concourse.mybir as mybir
    import concourse.tile as tile
    from concourse import bacc

    F32 = mybir.dt.float32
    BF16 = mybir.dt.bfloat16
    AF = mybir.ActivationFunctionType
    OP = mybir.AluOpType

    nc = bacc.Bacc("TRN2", target_bir_lowering=False, debug=False,
                   num_devices=NC)
    RG = [list(range(NC))]

    # ---------------- external parameters (per-core shards) ---------------
    ext = {}
    def inp(name, shape, dt=F32):
        ext[name] = nc.dram_tensor(name, shape, dt, kind="ExternalInput")
        return ext[name]

    embT = inp("embT", [128, DCH, TLOC])
    posT = inp("posT", [128, DCH, TLOC])
    # batched-pair weight layouts (pairs of 128-col chunks per DMA)
    wq = inp("wq", [n_layers, 4, 128, 2, DCH, 128], BF16)
    wk = inp("wk", [n_layers, 4, 128, 2, DCH, 128], BF16)
    wv = inp("wv", [n_layers, 4, 128, 2, DCH, 128], BF16)
    wo = inp("wo", [n_layers, 128, DCH, D], BF16)
    w1 = inp("w1", [n_layers, 16, 128, 2, DCH, 128], BF16)
    w2 = inp("w2", [n_layers, DCH, 128, 32, 128], BF16)
    wout = inp("wout", [16, 128, 2, DCH, VMC], BF16)
    bqkv = inp("bqkv", [128, DCH, 3, n_layers])
    boT = inp("boT", [128, DCH, n_layers])
    b1T = inp("b1T", [128, 32, n_layers])
    b2T = inp("b2T", [128, DCH, n_layers])
    boutT = inp("boutT", [VMC, VCH])
    g1T = inp("g1T", [128, DCH, n_layers])
    be1T = inp("be1T", [128, DCH, n_layers])
    g2T = inp("g2T", [128, DCH, n_layers])
    be2T = inp("be2T", [128, DCH, n_layers])
    gfT = inp("gfT", [128, DCH, 1])
    befT = inp("befT", [128, DCH, 1])
    cmask = inp("cmask", [128, 2, TLOC], BF16)
    ident = inp("ident", [128, 128], BF16)

    outT = nc.dram_tensor("outT", [VS, T], BF16, kind="ExternalOutput")

    dbg = {}
    def dbg_out(name, shape, dt=F32):
        if debug:
            dbg[name] = nc.dram_tensor(name, shape, dt, kind="ExternalOutput")
        return dbg.get(name)

    # ---------------- internal DRAM (collective bounce buffers) -----------
    HD2 = DCH // 2
    ag_in = nc.dram_tensor("ag0_in", [128, DCH, TLOC], BF16)
    ag_out = nc.dram_tensor("ag0_out", [NC * 128, DCH, TLOC], BF16,
                            addr_space="Shared")
    warm_in = nc.dram_tensor("warm_in", [128, 16], BF16)
    warm_out = nc.dram_tensor("warm_out", [NC * 128, 16], BF16,
                              addr_space="Shared")
    a2a1_in, a2a1_out = [], []
    aqkv_in, aqkv_out = [], []
    for i in range(n_layers):
        aqkv_in.append([nc.dram_tensor(f"aqkv{i}{j}_in",
                                       [NC, 128 * (2 - j), TLOC], BF16)
                        for j in range(2)])
        aqkv_out.append([nc.dram_tensor(f"aqkv{i}{j}_out",
                                        [NC, 128 * (2 - j), TLOC], BF16)
                         for j in range(2)])
        a2a1_in.append([nc.dram_tensor(f"a2a1{i}{h}_in", [NC, 64, TLOC], BF16)
                        for h in range(2)])
        a2a1_out.append([nc.dram_tensor(f"a2a1{i}{h}_out", [NC, 64, TLOC],
                                        BF16) for h in range(2)])

    from contextlib import ExitStack
    with tile.TileContext(nc) as tc, ExitStack() as stack:
            pool = lambda *a, **k: stack.enter_context(tc.tile_pool(*a, **k))
            konst = pool(name="const", bufs=1)
            resid = pool(name="resid", bufs=1)
            acts = pool(name="acts", bufs=1)
            wqkv_p = pool(name="wqkv", bufs=4)
            wop = pool(name="wop", bufs=4)
            w2p = pool(name="w2p", bufs=3)
            woutp = pool(name="woutp", bufs=3)
            lnp = pool(name="lnp", bufs=1)
            otp = pool(name="otp", bufs=2)
            att = pool(name="att", bufs=1)
            tmp = pool(name="tmp", bufs=3)
            expp = pool(name="expp", bufs=3)
            # PSUM is 8 banks statically partitioned: psA 4 x 1-bank slots
            # (score tiles / logits accumulators), psB 3 x 1-bank slots
            # (matmul accumulators / attention av / transposes), 1 stat bank.
            psA = pool(name="psA", bufs=4, space="PSUM")
            psB = pool(name="psB", bufs=3, space="PSUM")
            ps1 = pool(name="ps1", bufs=1, space="PSUM")

            # ---- constants resident in SBUF ----
            msk = konst.tile([128, 2, TLOC], BF16)
            nc.sync.dma_start(out=msk[:], in_=cmask[:, :, :])
            idn = konst.tile([128, 128], BF16)
            nc.sync.dma_start(out=idn[:], in_=ident[:, :])
            ones_bf = konst.tile([128, 1], BF16)
            nc.vector.memset(ones_bf[:], 1.0)
            eps_sb = konst.tile([1, 1], F32)
            nc.vector.memset(eps_sb[:], EPS)
            warm = konst.tile([1, 1], F32)
            nc.vector.memset(warm[:], 1.0)
            bqkv_sb = konst.tile([128, DCH, 3, n_layers], F32)
            nc.sync.dma_start(out=bqkv_sb[:], in_=bqkv[:, :, :])
            bo_sb = konst.tile([128, DCH, n_layers], F32)
            nc.sync.dma_start(out=bo_sb[:], in_=boT[:, :, :])
            b1_sb = konst.tile([128, 32, n_layers], F32)
            nc.sync.dma_start(out=b1_sb[:], in_=b1T[:, :, :])
            b2_sb = konst.tile([128, DCH, n_layers], F32)
            nc.sync.dma_start(out=b2_sb[:], in_=b2T[:, :, :])
            lng = {}
            for nm, t_ in (("g1", g1T), ("be1", be1T), ("g2", g2T),
                           ("be2", be2T), ("gf", gfT), ("bef", befT)):
                nl = 1 if nm in ("gf", "bef") else n_layers
                lnt = konst.tile([128, DCH, nl], F32, tag=f"ln_{nm}",
                                 name=f"ln_{nm}")
                nc.sync.dma_start(out=lnt[:], in_=t_[:, :, :])
                lng[nm] = lnt

            def prewarm(func):
                nc.scalar.activation(warm[:], warm[:], func)

            # ---- warm up ncfw/collective path early ----
            wtile = konst.tile([128, 16], BF16)
            nc.vector.memset(wtile[:], 0.0)
            nc.gpsimd.dma_start(out=warm_in[:, :], in_=wtile[:])
            nc.gpsimd.collective_compute(
                "AllGather", OP.bypass, replica_groups=RG,
                ins=[warm_in.ap().opt()], outs=[warm_out.ap().opt()])

            # ---- residual init: rT = embT + posT ----
            rT = resid.tile([128, DCH, TLOC], F32)
            pt = acts.tile([128, DCH, TLOC], F32, tag="scratch8k")
            nc.sync.dma_start(out=rT[:], in_=embT[:, :, :])
            nc.sync.dma_start(out=pt[:], in_=posT[:, :, :])
            nc.vector.tensor_add(rT[:], rT[:], pt[:])
            if debug:
                o = dbg_out("dbg_rT0", [128, DCH, TLOC])
                nc.sync.dma_start(out=o[:, :, :], in_=rT[:])

            # ------------- layernorm machinery (interleaved stats) --------
            class LN:
                def begin(self):
                    self.xs = lnp.tile([128, 2, DCH, TLOC], BF16,
                                       tag="ln_xs", name="ln_xs")
                    self.st = ps1.tile([1, 2, TLOC], F32, tag="pstat",
                                       name="ln_st")

                def chunk(self, c):
                    nc.vector.tensor_copy(self.xs[:, 0, c, :], rT[:, c, :])
                    nc.vector.tensor_mul(self.xs[:, 1, c, :], rT[:, c, :],
                                         rT[:, c, :])
                    nc.tensor.matmul(self.st[:], ones_bf[:],
                                     self.xs[:, :, c, :],
                                     start=(c == 0), stop=(c == DCH - 1))

                def finish(self, gname, bname, li, triv, site):
                    st = self.st
                    m1 = tmp.tile([1, TLOC], F32, tag="ln_m1", name="m1")
                    nc.vector.tensor_scalar_mul(m1[:], st[:, 0, :], 1.0 / D)
                    t2 = tmp.tile([1, TLOC], F32, tag="ln_t2", name="t2")
                    nc.vector.tensor_mul(t2[:], m1[:], m1[:])
                    v_ = tmp.tile([1, TLOC], F32, tag="ln_v", name="v_")
                    nc.vector.scalar_tensor_tensor(
                        v_[:], st[:, 1, :], 1.0 / D, t2[:],
                        op0=OP.mult, op1=OP.subtract)
                    nc.scalar.activation(t2[:], v_[:], AF.Sqrt,
                                         bias=eps_sb[:])
                    rstd = v_
                    nc.vector.reciprocal_approx_fast(out=rstd[:], in_=t2[:])
                    off = t2
                    nc.vector.scalar_tensor_tensor(
                        off[:], m1[:], -1.0, rstd[:],
                        op0=OP.mult, op1=OP.mult)
                    rstd_b = lnp.tile([128, TLOC], F32, tag="ln_rb",
                                      name="rb")
                    nc.gpsimd.partition_broadcast(rstd_b[:], rstd[:],
                                                  channels=128)
                    off_b = lnp.tile([128, TLOC], F32, tag="ln_ob", name="ob")
                    nc.gpsimd.partition_broadcast(off_b[:], off[:],
                                                  channels=128)
                    xo = lnp.tile([128, DCH, TLOC], BF16, tag="ln_out",
                                  name="xo")
                    rb3 = rstd_b[:, None, :].to_broadcast([128, HD2, TLOC])
                    ob3 = off_b[:, None, :].to_broadcast([128, HD2, TLOC])
                    self.xo = xo
                    for hh in range(2):
                        cs = slice(HD2 * hh, HD2 * (hh + 1))
                        if triv:
                            t_ = acts.tile([128, HD2, TLOC], F32,
                                           tag="scratch8k", name="lt")
                            nc.vector.tensor_mul(t_[:], rT[:, cs, :], rb3)
                            nc.vector.tensor_add(xo[:, cs, :], t_[:], ob3)
                        else:
                            for c in range(HD2 * hh, HD2 * (hh + 1)):
                                tc_ = lnp.tile([128, TLOC], F32,
                                               tag="ln_tc", name="ltc")
                                nc.vector.tensor_mul(tc_[:], rT[:, c, :],
                                                     rstd_b[:])
                                nc.vector.tensor_add(tc_[:], tc_[:],
                                                     off_b[:])
                                nc.vector.tensor_scalar(
                                    xo[:, c, :], tc_[:],
                                    lng[gname][:, c, li:li + 1],
                                    lng[bname][:, c, li:li + 1],
                                    op0=OP.mult, op1=OP.add)
                        if site is not None:
                            nc.gpsimd.dma_start(out=ag_in[:, cs, :],
                                                in_=xo[:, cs, :])
                    if site is not None:
                        nc.gpsimd.collective_compute(
                            "AllGather", OP.bypass, replica_groups=RG,
                            ins=[ag_in.ap().opt()],
                            outs=[ag_out.ap().opt()])

            ln = LN()
            prewarm(AF.Sqrt)

            # LN1 of layer 0 (stats over freshly built rT)
            ln.begin()
            for c in range(DCH):
                ln.chunk(c)
            ln.finish("g1", "be1", 0, ln_triv[0], None)
            prewarm(AF.Exp)

            # =================== layers ===================
            for li in range(n_layers):
                # ---- QKV computed on LOCAL tokens with full weights;
                #      q+k merged AllToAll, then v AllToAll, pipelined ----
                xo = ln.xo
                qT = acts.tile([128, T], BF16, tag="qT")
                kT = acts.tile([128, T], BF16, tag="kT")
                vT = acts.tile([128, T], BF16, tag="vT")
                qkl = att.tile([128, 2, DCH, TLOC], BF16, tag="qkvloc",
                               name="qkl")
                for j, wext in enumerate((wq, wk, wv)):
                    for q4 in range(4):
                        wt = wqkv_p.tile([128, 2, DCH, 128], BF16,
                                         tag="wqkv", name=f"wt{j}{q4}")
                        nc.sync.dma_start(out=wt[:], in_=wext[li, q4])
                        for ci in range(2):
                            cc = 2 * q4 + ci
                            pp = psB.tile([128, TLOC], F32, tag="b",
                                              name=f"qp{j}{cc}")
                            for d in range(DCH):
                                nc.tensor.matmul(pp[:], wt[:, ci, d, :],
                                                 xo[:, d, :],
                                                 start=(d == 0),
                                                 stop=(d == DCH - 1))
                            dstl = qkl[:, min(j, 1), cc, :]
                            nc.vector.tensor_scalar(
                                dstl, pp[:],
                                bqkv_sb[:, cc, j, li:li + 1], None,
                                op0=OP.add)
                    if j == 1:
                        # merged q+k AllToAll
                        for jj in range(2):
                            nc.gpsimd.dma_start(
                                out=aqkv_in[li][0].ap().rearrange(
                                    "r (j p) t -> p j r t", p=128)[:, jj],
                                in_=qkl[:, jj, :, :])
                        nc.gpsimd.collective_compute(
                            "AllToAll", OP.bypass, replica_groups=RG,
                            ins=[aqkv_in[li][0].ap().opt()],
                            outs=[aqkv_out[li][0].ap().opt()])
                        vv = aqkv_out[li][0].ap().rearrange(
                            "r (j p) t -> p j r t", p=128)
                        nc.gpsimd.dma_start(
                            out=qT[:].rearrange("p (r t) -> p r t", r=NC),
                            in_=vv[:, 0, :, :])
                        nc.gpsimd.dma_start(
                            out=kT[:].rearrange("p (r t) -> p r t", r=NC),
                            in_=vv[:, 1, :, :])
                    elif j == 2:
                        nc.gpsimd.dma_start(
                            out=aqkv_in[li][1].ap().rearrange(
                                "r p t -> p r t"),
                            in_=qkl[:, 1, :, :])
                        nc.gpsimd.collective_compute(
                            "AllToAll", OP.bypass, replica_groups=RG,
                            ins=[aqkv_in[li][1].ap().opt()],
                            outs=[aqkv_out[li][1].ap().opt()])
                        nc.gpsimd.dma_start(
                            out=vT[:].rearrange("p (r t) -> p r t", r=NC),
                            in_=aqkv_out[li][1].ap().rearrange(
                                "r p t -> p r t"))
                v_aug = acts.tile([128, 16, 130], BF16, tag="vaug")
                nc.vector.memset(v_aug[:, :, 64:65], 1.0)
                nc.vector.memset(v_aug[:, :, 129:130], 1.0)
                for tt in range(16):
                    tp = psB.tile([128, 128], BF16, tag="b", name="tp")
                    nc.tensor.transpose(tp[:], vT[:, 128 * tt:128 * (tt + 1)],
                                        idn[:])
                    nc.vector.tensor_copy(v_aug[:, tt, 0:64], tp[:, 0:64])
                    nc.vector.tensor_copy(v_aug[:, tt, 65:129], tp[:, 64:128])
                if debug and li == 0:
                    for nm, t_ in (("dbg_qT", qT), ("dbg_kT", kT),
                                   ("dbg_vaug", v_aug)):
                        o = dbg_out(nm, list(t_.shape), BF16)
                        nc.sync.dma_start(out=o.ap().opt(), in_=t_[:])

                # ---- attention, key-block-outer; per-head AllToAll ----
                # av [65, 1024] accumulates (out | denom) over key blocks;
                # one exp per key block over the full valid query range.
                hoT = acts.tile([64, 2, T], BF16, tag="hoT")
                for h in range(2):
                    hb = 64 * h
                    for b in range(B):
                        base = 1024 * b
                        avs = [psB.tile([65, 512], F32, tag="b",
                                        name=f"av{h}{b}{k}")
                               for k in range(2)]
                        for kc in range(8):
                            q0 = 128 * kc
                            segs = ([(q0, 512), (512, 1024)] if kc < 4
                                    else [(q0, 1024)])
                            for lo, hi in segs:
                                w_ = hi - lo
                                sc = psA.tile([128, 512], F32, tag="a",
                                              name=f"sc{h}{b}{kc}{lo}")
                                nc.tensor.matmul(
                                    sc[:, 0:w_],
                                    kT[hb:hb + 64, base + q0:base + q0 + 128],
                                    qT[hb:hb + 64, base + lo:base + hi],
                                    start=True, stop=True)
                                et_ = expp.tile([128, 512], BF16, tag="exp",
                                                name=f"et{h}{b}{kc}{lo}")
                                nc.scalar.activation(et_[:, 0:w_],
                                                     sc[:, 0:w_], AF.Exp,
                                                     scale=float(HD) ** -0.5)
                                if lo == q0:
                                    # diagonal block: mask keys > query
                                    nc.vector.tensor_mul(
                                        et_[:, 0:128], et_[:, 0:128],
                                        msk[:, 0, 0:128])
                                ah = 0 if hi <= 512 else 1
                                alo = lo - 512 * ah
                                nc.tensor.matmul(
                                    avs[ah][:, alo:alo + w_],
                                    v_aug[:, 8 * b + kc, 65 * h:65 * (h + 1)],
                                    et_[:, 0:w_],
                                    start=(kc == 0),
                                    stop=(kc == (3 if ah == 0 else 7)))
                        # evacuate: hoT rows + denominators
                        dnc = att.tile([1, 1024], F32, tag="dnc",
                                       name=f"dnc{h}{b}")
                        hot_b = att.tile([64, 1024], F32, tag="hotb",
                                         name=f"hotb{h}{b}")
                        for k in range(2):
                            nc.vector.tensor_copy(dnc[:, 512 * k:512 * (k + 1)],
                                                  avs[k][64:65, :])
                            nc.vector.tensor_copy(hot_b[:, 512 * k:512 * (k + 1)],
                                                  avs[k][0:64, :])
                        rc2 = att.tile([1, 1024], F32, tag="rc2",
                                       name=f"rc2{h}{b}")
                        nc.vector.reciprocal_approx_fast(out=rc2[:],
                                                         in_=dnc[:])
                        rcb = att.tile([64, 1024], F32, tag="rcb",
                                       name=f"rcb{h}{b}")
                        nc.gpsimd.partition_broadcast(rcb[:], rc2[:],
                                                      channels=64)
                        nc.vector.tensor_mul(
                            hoT[:, h, base:base + 1024], hot_b[:], rcb[:])
                    nc.gpsimd.dma_start(
                        out=a2a1_in[li][h].ap().rearrange("r p t -> p r t"),
                        in_=hoT[:, h, :].rearrange("p (r t) -> p r t", r=NC))
                    nc.gpsimd.collective_compute(
                        "AllToAll", OP.bypass, replica_groups=RG,
                        ins=[a2a1_in[li][h].ap().opt()],
                        outs=[a2a1_out[li][h].ap().opt()])
                prewarm(AF.Sqrt)
                if debug and li == 0:
                    o = dbg_out("dbg_hoT", [64, 2, T], BF16)
                    nc.sync.dma_start(out=o[:, :, :], in_=hoT[:])

                # ---- Wo (head-major rearranged rows) + LN2 stats hooks ----
                acs = []
                for h in range(2):
                    ach = acts.tile([128, 4, TLOC], BF16,
                                    tag=f"hoac{h}", name=f"ac{h}")
                    nc.gpsimd.dma_start(
                        out=ach[:],
                        in_=a2a1_out[li][h].ap().rearrange(
                            "(j k) p t -> (k p) j t", k=2))
                    acs.append(ach)
                ln.begin()
                wo_sbs = []
                for q4 in range(4):
                    wo_sb = wop.tile([128, DCH, 256], BF16, tag="wo",
                                     name=f"wo_sb{q4}")
                    nc.scalar.dma_start(
                        out=wo_sb[:],
                        in_=wo[li, :, :, 256 * q4:256 * (q4 + 1)])
                    wo_sbs.append(wo_sb)
                    for mi in range(2):
                        m = 2 * q4 + mi
                        pp = psB.tile([128, TLOC], F32, tag="b",
                                          name=f"wop_ps{m}")
                        for kc in range(4):
                            nc.tensor.matmul(
                                pp[:], wo_sb[:, kc, 128 * mi:128 * (mi + 1)],
                                acs[0][:, kc, :],
                                start=(kc == 0), stop=(kc == 3))
                        nc.vector.scalar_tensor_tensor(
                            rT[:, m, :], pp[:], bo_sb[:, m, li:li + 1],
                            rT[:, m, :], op0=OP.add, op1=OP.add)
                for q4 in range(4):
                    for mi in range(2):
                        m = 2 * q4 + mi
                        pp = psB.tile([128, TLOC], F32, tag="b",
                                          name=f"wop_ps2{m}")
                        for kc in range(4):
                            nc.tensor.matmul(
                                pp[:],
                                wo_sbs[q4][:, 4 + kc,
                                           128 * mi:128 * (mi + 1)],
                                acs[1][:, kc, :],
                                start=(kc == 0), stop=(kc == 3))
                        nc.vector.tensor_add(rT[:, m, :], rT[:, m, :], pp[:])
                        ln.chunk(m)
                if debug and li == 0:
                    o = dbg_out("dbg_rT1", [128, DCH, TLOC])
                    nc.sync.dma_start(out=o[:, :, :], in_=rT[:])
                ln.finish("g2", "be2", li, ln_triv[2 * li + 1], None)
                prewarm(AF.Gelu)

                # ---- FFN fully local: W1 full + gelu -> W2 full ----
                xo2 = ln.xo
                h1 = acts.tile([128, 32, TLOC], BF16, tag="bigact",
                               name="h1")
                for q16 in range(16):
                    wt1 = wqkv_p.tile([128, 2, DCH, 128], BF16, tag="wqkv",
                                      name=f"wt1{q16}")
                    nc.sync.dma_start(out=wt1[:], in_=w1[li, q16])
                    for ci in range(2):
                        cc = 2 * q16 + ci
                        pp = psB.tile([128, TLOC], F32, tag="b",
                                          name=f"w1p{cc}")
                        for d in range(DCH):
                            nc.tensor.matmul(pp[:], wt1[:, ci, d, :],
                                             xo2[:, d, :],
                                             start=(d == 0),
                                             stop=(d == DCH - 1))
                        nc.scalar.activation(h1[:, cc, :], pp[:], AF.Gelu,
                                             bias=b1_sb[:, cc, li:li + 1])
                prewarm(AF.Sqrt)
                if debug and li == 0:
                    o = dbg_out("dbg_h1T", [128, 32, TLOC], BF16)
                    nc.sync.dma_start(out=o[:, :, :], in_=h1[:])

                # ---- full W2 + residual + next-LN stats hooks ----
                ln.begin()
                for m in range(DCH):
                    w2_sb = w2p.tile([128, 32, 128], BF16, tag="w2",
                                     name=f"w2_sb{m}")
                    nc.scalar.dma_start(out=w2_sb[:], in_=w2[li, m, :, :, :])
                    pp = psB.tile([128, TLOC], F32, tag="b",
                                      name=f"w2_ps{m}")
                    for kc in range(32):
                        nc.tensor.matmul(pp[:], w2_sb[:, kc, :],
                                         h1[:, kc, :],
                                         start=(kc == 0), stop=(kc == 31))
                    nc.vector.scalar_tensor_tensor(
                        rT[:, m, :], pp[:], b2_sb[:, m, li:li + 1],
                        rT[:, m, :], op0=OP.add, op1=OP.add)
                    ln.chunk(m)
                if debug and li == 0:
                    o = dbg_out("dbg_rT2", [128, DCH, TLOC])
                    nc.sync.dma_start(out=o[:, :, :], in_=rT[:])
                if li + 1 < n_layers:
                    ln.finish("g1", "be1", li + 1, ln_triv[2 * (li + 1)],
                              None)
                    prewarm(AF.Exp)
                else:
                    ln.finish("gf", "bef", 0, ln_triv[2 * n_layers], 0)

            # =================== logits ===================
            # gathered activations -> xtf [128, NC, DCH, TLOC]
            xtf = acts.tile([128, NC, DCH, TLOC], BF16, tag="bigact",
                            name="xtf")
            agv = ag_out.ap().rearrange("(r p) c t -> p r c t", p=128)
            for q in range(4):
                nc.gpsimd.dma_start(
                    out=xtf[:, 2 * q:2 * q + 2, :, :],
                    in_=agv[:, 2 * q:2 * q + 2, :, :])
            bout_sb = None
            if use_bout:
                bout_sb = konst.tile([VMC, VCH], F32)
                nc.sync.dma_start(out=bout_sb[:], in_=boutT[:, :])
            for q2 in range(16):
                wo_t = woutp.tile([128, 2, DCH, VMC], BF16, tag="wout",
                                  name=f"wo_t{q2}")
                nc.sync.dma_start(out=wo_t[:], in_=wout[q2])
                for mi in range(2):
                    m = 2 * q2 + mi
                    ot = otp.tile([VMC, T], BF16, tag="ot", name=f"ot{m}")
                    pps = [psA.tile([VMC, 512], F32, tag="a",
                                    name=f"plga{m}_{k}") for k in range(2)] + \
                          [psB.tile([VMC, 512], F32, tag="b",
                                    name=f"plgb{m}_{k}") for k in range(2)]
                    for d in range(DCH):
                        for n in range(4):
                            nc.tensor.matmul(pps[n][:], wo_t[:, mi, d, :],
                                             xtf[:, 2 * n:2 * n + 2, d, :],
                                             start=(d == 0),
                                             stop=(d == DCH - 1))
                    for n in range(4):
                        if use_bout:
                            nc.vector.tensor_scalar(
                                ot[:, 512 * n:512 * (n + 1)], pps[n][:],
                                bout_sb[:, m:m + 1], None, op0=OP.add)
                        else:
                            nc.vector.tensor_copy(
                                ot[:, 512 * n:512 * (n + 1)], pps[n][:])
                    nc.scalar.dma_start(out=outT[VMC * m:VMC * (m + 1), :],
                                        in_=ot[:])

    nc.compile()
    return nc, sorted(dbg.keys())


def _prep_inputs(inputs):
    """Shard + transform full inputs -> list of 8 per-core in_maps."""
    f32 = lambda a: np.ascontiguousarray(np.asarray(a), dtype=np.float32)
    bf16 = lambda a: np.ascontiguousarray(np.asarray(a, dtype=np.float32)).astype(BF)

    x = np.asarray(inputs["x"]).astype(np.int64).reshape(T)
    tok = f32(inputs["tok_emb"])
    pos = f32(inputs["pos_emb"])
    Wq, Wk, Wv, Wo = (f32(inputs[k]) for k in ("Wq", "Wk", "Wv", "Wo"))
    W1, W2, Wout = f32(inputs["W1"]), f32(inputs["W2"]), f32(inputs["Wout"])
    bq, bk, bv = f32(inputs["bq"]), f32(inputs["bk"]), f32(inputs["bv"])
    bo, b1, b2 = f32(inputs["bo"]), f32(inputs["b1"]), f32(inputs["b2"])
    bout = f32(inputs["bout"])
    ln1_g, ln1_b = f32(inputs["ln1_g"]), f32(inputs["ln1_b"])
    ln2_g, ln2_b = f32(inputs["ln2_g"]), f32(inputs["ln2_b"])
    lnf_g, lnf_b = f32(inputs["lnf_g"]), f32(inputs["lnf_b"])

    def chunkT(vec_1d):  # [D] -> [128, DCH] (p, chunk)
        return np.ascontiguousarray(vec_1d.reshape(DCH, 128).T)

    # causal mask tiles: keep if f >= p + 128*r
    p_i = np.arange(128)[:, None]
    f_i = np.arange(TLOC)[None, :]
    cm = np.stack([(f_i >= p_i), (f_i >= p_i + 128)], 1).astype(BF)  # [128,2,256]
    idn = np.eye(128, dtype=BF)

    ln_triv = []
    for g, b in ((ln1_g, ln1_b), (ln2_g, ln2_b)):
        for i in range(L):
            ln_triv.append(bool(np.all(g[i] == 1.0) and np.all(b[i] == 0.0)))
    # interleave per layer: [ln1_0, ln2_0, ln1_1, ln2_1, ...]
    ln_triv = [ln_triv[i] if s == 0 else ln_triv[L + i]
               for i in range(L) for s in range(2)] + \
              [bool(np.all(lnf_g == 1.0) and np.all(lnf_b == 0.0))]
    use_bout = bool(np.any(bout))

    # [L, D, Do] -> [L, Do/256, 128(in-sub), 2(pair), DCH(in-chunk), 128]
    def qlay(w, do):
        a = w.reshape(L, DCH, 128, do // 128, 128).transpose(0, 3, 2, 1, 4)
        a = a.reshape(L, do // 256, 2, 128, DCH, 128).transpose(0, 1, 3, 2, 4, 5)
        return np.ascontiguousarray(a)

    in_maps = []
    for c in range(NC):
        xl = x[TLOC * c:TLOC * (c + 1)]
        embT_ = np.ascontiguousarray(tok[xl].T).reshape(DCH, 128, TLOC)
        embT_ = np.ascontiguousarray(embT_.transpose(1, 0, 2))  # [128, DCH, 256]
        p0 = (c % 4) * TLOC
        posT_ = np.ascontiguousarray(pos[p0:p0 + TLOC].T).reshape(DCH, 128, TLOC)
        posT_ = np.ascontiguousarray(posT_.transpose(1, 0, 2))
        lay = lambda w: np.ascontiguousarray(
            w.reshape(L, DCH, 128, w.shape[-1]).transpose(0, 2, 1, 3))
        w2h = np.ascontiguousarray(
            W2.reshape(L, 32, 128, DCH, 128).transpose(0, 3, 2, 1, 4))
        wouth = Wout[:, VS * c:VS * (c + 1)].reshape(DCH, 128, VCH, VMC) \
            .transpose(2, 1, 0, 3)  # [VCH, 128, DCH, VMC]
        wouth = wouth.reshape(16, 2, 128, DCH, VMC).transpose(0, 2, 1, 3, 4)
        wouth = np.ascontiguousarray(wouth)  # [16, 128, 2, DCH, VMC]
        m = dict(
            embT=embT_, posT=posT_,
            wq=bf16(qlay(Wq, D)), wk=bf16(qlay(Wk, D)), wv=bf16(qlay(Wv, D)),
            wo=bf16(lay(Wo.reshape(L, NC, 2, 64, D).transpose(0, 2, 1, 3, 4)
                        .reshape(L, D, D))),
            w1=bf16(qlay(W1, F)),
            w2=bf16(w2h), wout=bf16(wouth),
            bqkv=np.ascontiguousarray(
                np.stack([np.stack([chunkT(bb[i]) for i in range(L)], -1)
                          for bb in (bq, bk, bv)], 2)),  # [128,DCH,3,L]
            boT=np.ascontiguousarray(
                np.stack([chunkT(bo[i]) for i in range(L)], -1)),
            b1T=np.ascontiguousarray(np.stack(
                [np.ascontiguousarray(b1[i].reshape(32, 128).T)
                 for i in range(L)], -1)),
            b2T=np.ascontiguousarray(
                np.stack([chunkT(b2[i]) for i in range(L)], -1)),
            boutT=np.ascontiguousarray(
                bout[VS * c:VS * (c + 1)].reshape(VCH, VMC).T),
            g1T=np.ascontiguousarray(
                np.stack([chunkT(ln1_g[i]) for i in range(L)], -1)),
            be1T=np.ascontiguousarray(
                np.stack([chunkT(ln1_b[i]) for i in range(L)], -1)),
            g2T=np.ascontiguousarray(
                np.stack([chunkT(ln2_g[i]) for i in range(L)], -1)),
            be2T=np.ascontiguousarray(
                np.stack([chunkT(ln2_b[i]) for i in range(L)], -1)),
            gfT=chunkT(lnf_g)[:, :, None].copy(),
            befT=chunkT(lnf_b)[:, :, None].copy(),
            cmask=cm, ident=idn,
        )
        in_maps.append(m)
    return in_maps, use_bout, ln_triv


_CACHE = {}


def _get_nc(n_layers, use_bout, ln_triv, debug=False):
    key = (n_layers, use_bout, tuple(ln_triv), debug)
    if key not in _CACHE:
        _CACHE[key] = _build(n_layers, use_bout, ln_triv, debug)
    return _CACHE[key]


def run(inputs, n_layers=L, debug=False, trace=False):
    from concourse import bass_utils
    in_maps, use_bout, ln_triv = _prep_inputs(inputs)
    nc, dbg_names = _get_nc(n_layers, use_bout, ln_triv, debug)
    res = bass_utils.run_bass_kernel_spmd(
        nc, in_maps, core_ids=list(range(NC)), trace=trace)
    return res, dbg_names


def kernel(**inputs):
    res, _ = run(inputs)
    outs = [res.results[c]["outT"] for c in range(NC)]          # [4000, 2048] bf16
    logits = np.concatenate([np.asarray(o, np.float32).T for o in outs],
                            axis=1)                             # [2048, 32000]
    return np.ascontiguousarray(logits.reshape(B, S, V))


# revision 27
# speedup vs baseline: 1.1173x; 1.0032x over previous
"""Self-contained Trainium2 Bass kernel for a 3-layer dense transformer LM.

Model (fp32 reference): embed -> 3x[LN -> MHA(causal) -> +res -> LN -> FFN(gelu) -> +res]
-> LN -> logits.  B=2, S=1024, D=1024, H=16, HD=64, F=4096, V=32000.

Distribution over 8 NeuronCores (one TRN2 chip), Megatron-SP style with
AllToAll instead of reduce-scatter:
  - Residual stream is sequence-sharded: core c owns 256 tokens, kept
    TRANSPOSED in SBUF as rT [D=8x128 partitions-chunks, 256 tokens].
  - LN computed on local tokens (stats via ones-matmul over partition
    chunks).
  - Attention tensor-parallel over heads: core c computes heads 2c,2c+1
    for ALL tokens; per-head causal softmax without max subtraction
    (scores are small); denominators come free via a ones-column in v.
    Softmax is key-block-outer: one exp per key block over the full
    valid query range (few large ACT ops instead of many small ones).
  - AllToAll converts head-sharded attention output to token-sharded,
    then each core applies the FULL Wo for its own 256 tokens. Same
    pattern for FFN: full W1 + full W2 locally (no collective).
  - Logits: final LN -> AllGather -> each core computes a 4000-column
    vocab slice for all 2048 tokens; host concatenates.

Compute dtype bf16 (PE full rate), accumulation fp32 in PSUM, output
logits written bf16.  Weight streaming: 512KB-1MB batched DMAs split
across the two HWDGE rings (sync=qSP for QKV/W1/wout, scalar=qAct for
Wo/W2/outputs); collective staging on the gpsimd SWDGE ring.
"""

import numpy as np
import ml_dtypes

BF = ml_dtypes.bfloat16

B, S, D, H, L, F, V = 2, 1024, 1024, 16, 3, 4096, 32000
HD = D // H
T = B * S            # 2048 tokens
NC = 8               # cores
TLOC = T // NC       # 256 tokens per core
VS = V // NC         # 4000 vocab cols per core
EPS = 1e-5
DCH = D // 128       # 8 partition chunks of the hidden dim
VCH = 32             # vocab m-chunks per core
VMC = VS // VCH      # 125 vocab cols per m-chunk


def _build(n_layers, use_bout, ln_triv, debug=False):
    import concourse.bass as bass
    import concourse.mybir as mybir
    import concourse.tile as tile
    from concourse import bacc

    F32 = mybir.dt.float32
    BF16 = mybir.dt.bfloat16
    AF = mybir.ActivationFunctionType
    OP = mybir.AluOpType

    nc = bacc.Bacc("TRN2", target_bir_lowering=False, debug=False,
                   num_devices=NC)
    RG = [list(range(NC))]

    # ---------------- external parameters (per-core shards) ---------------
    ext = {}
    def inp(name, shape, dt=F32):
        ext[name] = nc.dram_tensor(name, shape, dt, kind="ExternalInput")
        return ext[name]

    embT = inp("embT", [128, DCH, TLOC])
    posT = inp("posT", [128, DCH, TLOC])
    # batched-pair weight layouts (pairs of 128-col chunks per DMA)
    wq = inp("wq", [n_layers, 4, 128, 2, DCH, 128], BF16)
    wk = inp("wk", [n_layers, 4, 128, 2, DCH, 128], BF16)
    wv = inp("wv", [n_layers, 4, 128, 2, DCH, 128], BF16)
    wo = inp("wo", [n_layers, 128, DCH, D], BF16)
    w1 = inp("w1", [n_layers, 16, 128, 2, DCH, 128], BF16)
    w2 = inp("w2", [n_layers, DCH, 128, 32, 128], BF16)
    wout = inp("wout", [16, 128, 2, DCH, VMC], BF16)
    bqkv = inp("bqkv", [128, DCH, 3, n_layers])
    boT = inp("boT", [128, DCH, n_layers])
    b1T = inp("b1T", [128, 32, n_layers])
    b2T = inp("b2T", [128, DCH, n_layers])
    boutT = inp("boutT", [VMC, VCH])
    g1T = inp("g1T", [128, DCH, n_layers])
    be1T = inp("be1T", [128, DCH, n_layers])
    g2T = inp("g2T", [128, DCH, n_layers])
    be2T = inp("be2T", [128, DCH, n_layers])
    gfT = inp("gfT", [128, DCH, 1])
    befT = inp("befT", [128, DCH, 1])
    cmask = inp("cmask", [128, 2, TLOC], BF16)
    ident = inp("ident", [128, 128], BF16)

    outT = nc.dram_tensor("outT", [VS, T], BF16, kind="ExternalOutput")

    dbg = {}
    def dbg_out(name, shape, dt=F32):
        if debug:
            dbg[name] = nc.dram_tensor(name, shape, dt, kind="ExternalOutput")
        return dbg.get(name)

    # ---------------- internal DRAM (collective bounce buffers) -----------
    HD2 = DCH // 2
    ag_in = [nc.dram_tensor(f"ag{k}_in", [128, HD2, TLOC], BF16)
             for k in range(2)]
    ag_out = [nc.dram_tensor(f"ag{k}_out", [NC * 128, HD2, TLOC], BF16,
                             addr_space="Shared") for k in range(2)]
    warm_in = nc.dram_tensor("warm_in", [128, 16], BF16)
    warm_out = nc.dram_tensor("warm_out", [NC * 128, 16], BF16,
                              addr_space="Shared")
    warm2_in = nc.dram_tensor("warm2_in", [NC, 16, 16], BF16)
    warm2_out = nc.dram_tensor("warm2_out", [NC, 16, 16], BF16)
    a2a1_in, a2a1_out = [], []
    aqkv_in, aqkv_out = [], []
    for i in range(n_layers):
        aqkv_in.append([nc.dram_tensor(f"aqkv{i}{j}_in",
                                       [NC, 128 * (2 - j), TLOC], BF16)
                        for j in range(2)])
        aqkv_out.append([nc.dram_tensor(f"aqkv{i}{j}_out",
                                        [NC, 128 * (2 - j), TLOC], BF16)
                         for j in range(2)])
        a2a1_in.append([nc.dram_tensor(f"a2a1{i}{h}_in", [NC, 64, TLOC], BF16)
                        for h in range(2)])
        a2a1_out.append([nc.dram_tensor(f"a2a1{i}{h}_out", [NC, 64, TLOC],
                                        BF16) for h in range(2)])

    from contextlib import ExitStack
    with tile.TileContext(nc) as tc, ExitStack() as stack:
            pool = lambda *a, **k: stack.enter_context(tc.tile_pool(*a, **k))
            konst = pool(name="const", bufs=1)
            resid = pool(name="resid", bufs=1)
            acts = pool(name="acts", bufs=1)
            wqkv_p = pool(name="wqkv", bufs=4)
            wop = pool(name="wop", bufs=4)
            w2p = pool(name="w2p", bufs=3)
            woutp = pool(name="woutp", bufs=3)
            lnp = pool(name="lnp", bufs=1)
            otp = pool(name="otp", bufs=2)
            att = pool(name="att", bufs=1)
            tmp = pool(name="tmp", bufs=3)
            expp = pool(name="expp", bufs=3)
            # PSUM is 8 banks statically partitioned: psA 2 x 2-bank slots
            # (score tiles / logits accumulators), psB 3 x 1-bank slots
            # (matmul accumulators / attention av / transposes), 1 stat bank.
            psA = pool(name="psA", bufs=2, space="PSUM")
            psB = pool(name="psB", bufs=3, space="PSUM")
            ps1 = pool(name="ps1", bufs=1, space="PSUM")

            # ---- constants resident in SBUF ----
            msk = konst.tile([128, 2, TLOC], BF16)
            nc.sync.dma_start(out=msk[:], in_=cmask[:, :, :])
            idn = konst.tile([128, 128], BF16)
            nc.sync.dma_start(out=idn[:], in_=ident[:, :])
            ones_bf = konst.tile([128, 1], BF16)
            nc.vector.memset(ones_bf[:], 1.0)
            eps_sb = konst.tile([1, 1], F32)
            nc.vector.memset(eps_sb[:], EPS)
            warm = konst.tile([1, 1], F32)
            nc.vector.memset(warm[:], 1.0)
            bqkv_sb = konst.tile([128, DCH, 3, n_layers], F32)
            nc.sync.dma_start(out=bqkv_sb[:], in_=bqkv[:, :, :])
            bo_sb = konst.tile([128, DCH, n_layers], F32)
            nc.sync.dma_start(out=bo_sb[:], in_=boT[:, :, :])
            b1_sb = konst.tile([128, 32, n_layers], F32)
            nc.sync.dma_start(out=b1_sb[:], in_=b1T[:, :, :])
            b2_sb = konst.tile([128, DCH, n_layers], F32)
            nc.sync.dma_start(out=b2_sb[:], in_=b2T[:, :, :])
            lng = {}
            for nm, t_ in (("g1", g1T), ("be1", be1T), ("g2", g2T),
                           ("be2", be2T), ("gf", gfT), ("bef", befT)):
                nl = 1 if nm in ("gf", "bef") else n_layers
                lnt = konst.tile([128, DCH, nl], F32, tag=f"ln_{nm}",
                                 name=f"ln_{nm}")
                nc.sync.dma_start(out=lnt[:], in_=t_[:, :, :])
                lng[nm] = lnt

            def prewarm(func):
                nc.scalar.activation(warm[:], warm[:], func)

            # ---- warm up ncfw/collective path early ----
            wtile = konst.tile([128, 16], BF16)
            nc.vector.memset(wtile[:], 0.0)
            nc.gpsimd.dma_start(out=warm_in[:, :], in_=wtile[:])
            nc.gpsimd.collective_compute(
                "AllGather", OP.bypass, replica_groups=RG,
                ins=[warm_in.ap().opt()], outs=[warm_out.ap().opt()])
            nc.gpsimd.dma_start(
                out=warm2_in.ap().rearrange("r p t -> (r p) t"),
                in_=wtile[:])
            nc.gpsimd.collective_compute(
                "AllToAll", OP.bypass, replica_groups=RG,
                ins=[warm2_in.ap().opt()], outs=[warm2_out.ap().opt()])

            # ---- residual init: rT = embT + posT ----
            rT = resid.tile([128, DCH, TLOC], F32)
            pt = acts.tile([128, DCH, TLOC], F32, tag="scratch8k")
            nc.sync.dma_start(out=rT[:], in_=embT[:, :, :])
            nc.sync.dma_start(out=pt[:], in_=posT[:, :, :])
            nc.vector.tensor_add(rT[:], rT[:], pt[:])
            if debug:
                o = dbg_out("dbg_rT0", [128, DCH, TLOC])
                nc.sync.dma_start(out=o[:, :, :], in_=rT[:])

            # ------------- layernorm machinery (interleaved stats) --------
            class LN:
                def begin(self):
                    self.xs = lnp.tile([128, 2, DCH, TLOC], BF16,
                                       tag="ln_xs", name="ln_xs")
                    self.st = ps1.tile([1, 2, TLOC], F32, tag="pstat",
                                       name="ln_st")

                def chunk(self, c):
                    nc.vector.tensor_copy(self.xs[:, 0, c, :], rT[:, c, :])
                    nc.vector.tensor_mul(self.xs[:, 1, c, :], rT[:, c, :],
                                         rT[:, c, :])
                    nc.tensor.matmul(self.st[:], ones_bf[:],
                                     self.xs[:, :, c, :],
                                     start=(c == 0), stop=(c == DCH - 1))

                def finish(self, gname, bname, li, triv, site):
                    st = self.st
                    m1 = tmp.tile([1, TLOC], F32, tag="ln_m1", name="m1")
                    nc.vector.tensor_scalar_mul(m1[:], st[:, 0, :], 1.0 / D)
                    t2 = tmp.tile([1, TLOC], F32, tag="ln_t2", name="t2")
                    nc.vector.tensor_mul(t2[:], m1[:], m1[:])
                    v_ = tmp.tile([1, TLOC], F32, tag="ln_v", name="v_")
                    nc.vector.scalar_tensor_tensor(
                        v_[:], st[:, 1, :], 1.0 / D, t2[:],
                        op0=OP.mult, op1=OP.subtract)
                    # rstd = (var+eps)^-0.5 via log/exp (stays in the exp
                    # table set -- avoids sqrt table swaps on ACT)
                    nc.scalar.activation(t2[:], v_[:], AF.Ln,
                                         bias=eps_sb[:])
                    rstd = v_
                    nc.scalar.activation(rstd[:], t2[:], AF.Exp, scale=-0.5)
                    off = t2
                    nc.vector.scalar_tensor_tensor(
                        off[:], m1[:], -1.0, rstd[:],
                        op0=OP.mult, op1=OP.mult)
                    rstd_b = lnp.tile([128, TLOC], F32, tag="ln_rb",
                                      name="rb")
                    nc.gpsimd.partition_broadcast(rstd_b[:], rstd[:],
                                                  channels=128)
                    off_b = lnp.tile([128, TLOC], F32, tag="ln_ob", name="ob")
                    nc.gpsimd.partition_broadcast(off_b[:], off[:],
                                                  channels=128)
                    xo = lnp.tile([128, DCH, TLOC], BF16, tag="ln_out",
                                  name="xo")
                    rb3 = rstd_b[:, None, :].to_broadcast([128, HD2, TLOC])
                    ob3 = off_b[:, None, :].to_broadcast([128, HD2, TLOC])
                    self.xo = xo
                    for hh in range(2):
                        cs = slice(HD2 * hh, HD2 * (hh + 1))
                        if triv:
                            t_ = acts.tile([128, HD2, TLOC], F32,
                                           tag="scratch8k", name="lt")
                            nc.vector.tensor_mul(t_[:], rT[:, cs, :], rb3)
                            nc.vector.tensor_add(xo[:, cs, :], t_[:], ob3)
                        else:
                            for c in range(HD2 * hh, HD2 * (hh + 1)):
                                tc_ = lnp.tile([128, TLOC], F32,
                                               tag="ln_tc", name="ltc")
                                nc.vector.tensor_mul(tc_[:], rT[:, c, :],
                                                     rstd_b[:])
                                nc.vector.tensor_add(tc_[:], tc_[:],
                                                     off_b[:])
                                nc.vector.tensor_scalar(
                                    xo[:, c, :], tc_[:],
                                    lng[gname][:, c, li:li + 1],
                                    lng[bname][:, c, li:li + 1],
                                    op0=OP.mult, op1=OP.add)
                        if site is not None:
                            # split AllGather: each feature-half gathers as
                            # soon as it is normalized, overlapping logits
                            nc.gpsimd.dma_start(out=ag_in[hh][:, :, :],
                                                in_=xo[:, cs, :])
                            nc.gpsimd.collective_compute(
                                "AllGather", OP.bypass, replica_groups=RG,
                                ins=[ag_in[hh].ap().opt()],
                                outs=[ag_out[hh].ap().opt()])

            ln = LN()
            prewarm(AF.Ln)

            # LN1 of layer 0 (stats over freshly built rT)
            ln.begin()
            for c in range(DCH):
                ln.chunk(c)
            ln.finish("g1", "be1", 0, ln_triv[0], None)
            prewarm(AF.Exp)

            # =================== layers ===================
            for li in range(n_layers):
                # ---- QKV computed on LOCAL tokens with full weights;
                #      q+k merged AllToAll, then v AllToAll, pipelined ----
                xo = ln.xo
                qT = acts.tile([128, T], BF16, tag="qT")
                kT = acts.tile([128, T], BF16, tag="kT")
                vT = acts.tile([128, T], BF16, tag="vT")
                qkl = att.tile([128, 2, DCH, TLOC], BF16, tag="qkvloc",
                               name="qkl")
                # v first: its AllToAll + transposes complete while q/k
                # project, so attention runs gapless once q+k land.
                for j, wext in ((2, wv), (0, wq), (1, wk)):
                    for q4 in range(4):
                        wt = wqkv_p.tile([128, 2, DCH, 128], BF16,
                                         tag="wqkv", name=f"wt{j}{q4}")
                        nc.sync.dma_start(out=wt[:], in_=wext[li, q4])
                        for ci in range(2):
                            cc = 2 * q4 + ci
                            pp = psB.tile([128, TLOC], F32, tag="b",
                                              name=f"qp{j}{cc}")
                            for d in range(DCH):
                                nc.tensor.matmul(pp[:], wt[:, ci, d, :],
                                                 xo[:, d, :],
                                                 start=(d == 0),
                                                 stop=(d == DCH - 1))
                            dstl = qkl[:, min(j, 1), cc, :]
                            nc.vector.tensor_scalar(
                                dstl, pp[:],
                                bqkv_sb[:, cc, j, li:li + 1], None,
                                op0=OP.add)
                    if j == 2:
                        # v AllToAll (issued first)
                        nc.gpsimd.dma_start(
                            out=aqkv_in[li][1].ap().rearrange(
                                "r p t -> p r t"),
                            in_=qkl[:, 1, :, :])
                        nc.gpsimd.collective_compute(
                            "AllToAll", OP.bypass, replica_groups=RG,
                            ins=[aqkv_in[li][1].ap().opt()],
                            outs=[aqkv_out[li][1].ap().opt()])
                        nc.scalar.dma_start(
                            out=vT[:].rearrange("p (r t) -> p r t", r=NC),
                            in_=aqkv_out[li][1].ap().rearrange(
                                "r p t -> p r t"))
                    elif j == 1:
                        # merged q+k AllToAll
                        for jj in range(2):
                            nc.gpsimd.dma_start(
                                out=aqkv_in[li][0].ap().rearrange(
                                    "r (j p) t -> p j r t", p=128)[:, jj],
                                in_=qkl[:, jj, :, :])
                        nc.gpsimd.collective_compute(
                            "AllToAll", OP.bypass, replica_groups=RG,
                            ins=[aqkv_in[li][0].ap().opt()],
                            outs=[aqkv_out[li][0].ap().opt()])
                        vv = aqkv_out[li][0].ap().rearrange(
                            "r (j p) t -> p j r t", p=128)
                        nc.scalar.dma_start(
                            out=qT[:].rearrange("p (r t) -> p r t", r=NC),
                            in_=vv[:, 0, :, :])
                        nc.scalar.dma_start(
                            out=kT[:].rearrange("p (r t) -> p r t", r=NC),
                            in_=vv[:, 1, :, :])
                v_aug = acts.tile([128, 16, 130], BF16, tag="vaug")
                nc.vector.memset(v_aug[:, :, 64:65], 1.0)
                nc.vector.memset(v_aug[:, :, 129:130], 1.0)
                for tt in range(16):
                    tp = psB.tile([128, 128], BF16, tag="b", name="tp")
                    nc.tensor.transpose(tp[:], vT[:, 128 * tt:128 * (tt + 1)],
                                        idn[:])
                    nc.vector.tensor_copy(v_aug[:, tt, 0:64], tp[:, 0:64])
                    nc.vector.tensor_copy(v_aug[:, tt, 65:129], tp[:, 64:128])
                if debug and li == 0:
                    for nm, t_ in (("dbg_qT", qT), ("dbg_kT", kT),
                                   ("dbg_vaug", v_aug)):
                        o = dbg_out(nm, list(t_.shape), BF16)
                        nc.sync.dma_start(out=o.ap().opt(), in_=t_[:])

                # ---- attention, key-block-outer; per-head AllToAll ----
                # av [65, 1024] accumulates (out | denom) over key blocks;
                # one exp per key block over the full valid query range.
                hoT = acts.tile([64, 2, T], BF16, tag="hoT")
                for h in range(2):
                    hb = 64 * h
                    for b in range(B):
                        base = 1024 * b
                        avs = [psB.tile([65, 512], F32, tag="b",
                                        name=f"av{h}{b}{k}")
                               for k in range(2)]
                        ets = {}

                        def emit_sc(kc):
                            q0 = 128 * kc
                            nq = 1024 - q0
                            # scores live at ABSOLUTE query offsets in the
                            # 2-bank tile so each MM stays within one bank
                            sc = psA.tile([128, 1024], F32, tag="a",
                                          name=f"sc{h}{b}{kc}")
                            segs = ([(q0, 512), (512, 1024)] if kc < 4
                                    else [(q0, 1024)])
                            for lo, hi in segs:
                                nc.tensor.matmul(
                                    sc[:, lo:hi],
                                    kT[hb:hb + 64, base + q0:base + q0 + 128],
                                    qT[hb:hb + 64, base + lo:base + hi],
                                    start=True, stop=True)
                            et_ = expp.tile([128, 1024], BF16, tag="exp",
                                            name=f"et{h}{b}{kc}")
                            nc.scalar.activation(et_[:, 0:nq], sc[:, q0:1024],
                                                 AF.Exp,
                                                 scale=float(HD) ** -0.5)
                            # diagonal block: mask keys > query
                            nc.vector.tensor_mul(et_[:, 0:128],
                                                 et_[:, 0:128],
                                                 msk[:, 0, 0:128])
                            ets[kc] = et_

                        def emit_av(kc):
                            q0 = 128 * kc
                            et_ = ets.pop(kc)
                            for ah in range(2):
                                lo = max(q0, 512 * ah)
                                hi = 512 * (ah + 1)
                                if lo >= hi:
                                    continue
                                nc.tensor.matmul(
                                    avs[ah][:, lo - 512 * ah:hi - 512 * ah],
                                    v_aug[:, 8 * b + kc, 65 * h:65 * (h + 1)],
                                    et_[:, lo - q0:hi - q0],
                                    start=(kc == 0),
                                    stop=(kc == (3 if ah == 0 else 7)))

                        # software pipeline: scores run one block ahead of
                        # the av accumulation so PE never waits on exp
                        emit_sc(0)
                        for kc in range(1, 8):
                            emit_sc(kc)
                            emit_av(kc - 1)
                        emit_av(7)
                        # evacuate: hoT rows + denominators
                        dnc = att.tile([1, 1024], F32, tag="dnc",
                                       name=f"dnc{h}{b}")
                        hot_b = att.tile([64, 1024], F32, tag="hotb",
                                         name=f"hotb{h}{b}")
                        for k in range(2):
                            nc.vector.tensor_copy(dnc[:, 512 * k:512 * (k + 1)],
                                                  avs[k][64:65, :])
                            nc.vector.tensor_copy(hot_b[:, 512 * k:512 * (k + 1)],
                                                  avs[k][0:64, :])
                        rc2 = att.tile([1, 1024], F32, tag="rc2",
                                       name=f"rc2{h}{b}")
                        nc.vector.reciprocal_approx_fast(out=rc2[:],
                                                         in_=dnc[:])
                        rcb = att.tile([64, 1024], F32, tag="rcb",
                                       name=f"rcb{h}{b}")
                        nc.gpsimd.partition_broadcast(rcb[:], rc2[:],
                                                      channels=64)
                        nc.vector.tensor_mul(
                            hoT[:, h, base:base + 1024], hot_b[:], rcb[:])
                        # stage this batch-half immediately
                        nc.gpsimd.dma_start(
                            out=a2a1_in[li][h].ap().rearrange(
                                "r p t -> p r t")[:, 4 * b:4 * (b + 1), :],
                            in_=hoT[:, h, base:base + 1024].rearrange(
                                "p (r t) -> p r t", r=4))
                    nc.gpsimd.collective_compute(
                        "AllToAll", OP.bypass, replica_groups=RG,
                        ins=[a2a1_in[li][h].ap().opt()],
                        outs=[a2a1_out[li][h].ap().opt()])
                prewarm(AF.Ln)
                if debug and li == 0:
                    o = dbg_out("dbg_hoT", [64, 2, T], BF16)
                    nc.sync.dma_start(out=o[:, :, :], in_=hoT[:])

                # ---- Wo (head-major rearranged rows) + LN2 stats hooks ----
                acs = []
                for h in range(2):
                    ach = acts.tile([128, 4, TLOC], BF16,
                                    tag=f"hoac{h}", name=f"ac{h}")
                    src = a2a1_out[li][h].ap().rearrange(
                        "(j k) p t -> (k p) j t", k=2)
                    nc.scalar.dma_start(out=ach[:, 0:2, :],
                                        in_=src[:, 0:2, :])
                    nc.scalar.dma_start(out=ach[:, 2:4, :],
                                        in_=src[:, 2:4, :])
                    acs.append(ach)
                ln.begin()
                wo_sbs = []
                for q4 in range(4):
                    wo_sb = wop.tile([128, DCH, 256], BF16, tag="wo",
                                     name=f"wo_sb{q4}")
                    nc.scalar.dma_start(
                        out=wo_sb[:],
                        in_=wo[li, :, :, 256 * q4:256 * (q4 + 1)])
                    wo_sbs.append(wo_sb)
                    for mi in range(2):
                        m = 2 * q4 + mi
                        pp = psB.tile([128, TLOC], F32, tag="b",
                                          name=f"wop_ps{m}")
                        for kc in range(4):
                            nc.tensor.matmul(
                                pp[:], wo_sb[:, kc, 128 * mi:128 * (mi + 1)],
                                acs[0][:, kc, :],
                                start=(kc == 0), stop=(kc == 3))
                        nc.vector.scalar_tensor_tensor(
                            rT[:, m, :], pp[:], bo_sb[:, m, li:li + 1],
                            rT[:, m, :], op0=OP.add, op1=OP.add)
                for q4 in range(4):
                    for mi in range(2):
                        m = 2 * q4 + mi
                        pp = psB.tile([128, TLOC], F32, tag="b",
                                          name=f"wop_ps2{m}")
                        for kc in range(4):
                            nc.tensor.matmul(
                                pp[:],
                                wo_sbs[q4][:, 4 + kc,
                                           128 * mi:128 * (mi + 1)],
                                acs[1][:, kc, :],
                                start=(kc == 0), stop=(kc == 3))
                        nc.vector.tensor_add(rT[:, m, :], rT[:, m, :], pp[:])
                        ln.chunk(m)
                if debug and li == 0:
                    o = dbg_out("dbg_rT1", [128, DCH, TLOC])
                    nc.sync.dma_start(out=o[:, :, :], in_=rT[:])
                ln.finish("g2", "be2", li, ln_triv[2 * li + 1], None)
                prewarm(AF.Gelu)

                # ---- FFN fully local: W1 full + gelu -> W2 full ----
                xo2 = ln.xo
                h1 = acts.tile([128, 32, TLOC], BF16, tag="bigact",
                               name="h1")
                for q16 in range(16):
                    wt1 = wqkv_p.tile([128, 2, DCH, 128], BF16, tag="wqkv",
                                      name=f"wt1{q16}")
                    nc.sync.dma_start(out=wt1[:], in_=w1[li, q16])
                    for ci in range(2):
                        cc = 2 * q16 + ci
                        pp = psB.tile([128, TLOC], F32, tag="b",
                                          name=f"w1p{cc}")
                        for d in range(DCH):
                            nc.tensor.matmul(pp[:], wt1[:, ci, d, :],
                                             xo2[:, d, :],
                                             start=(d == 0),
                                             stop=(d == DCH - 1))
                        nc.scalar.activation(h1[:, cc, :], pp[:], AF.Gelu,
                                             bias=b1_sb[:, cc, li:li + 1])
                prewarm(AF.Ln)
                if debug and li == 0:
                    o = dbg_out("dbg_h1T", [128, 32, TLOC], BF16)
                    nc.sync.dma_start(out=o[:, :, :], in_=h1[:])

                # ---- full W2 + residual + next-LN stats hooks ----
                ln.begin()
                for m in range(DCH):
                    w2_sb = w2p.tile([128, 32, 128], BF16, tag="w2",
                                     name=f"w2_sb{m}")
                    nc.scalar.dma_start(out=w2_sb[:], in_=w2[li, m, :, :, :])
                    pp = psB.tile([128, TLOC], F32, tag="b",
                                      name=f"w2_ps{m}")
                    for kc in range(32):
                        nc.tensor.matmul(pp[:], w2_sb[:, kc, :],
                                         h1[:, kc, :],
                                         start=(kc == 0), stop=(kc == 31))
                    nc.vector.scalar_tensor_tensor(
                        rT[:, m, :], pp[:], b2_sb[:, m, li:li + 1],
                        rT[:, m, :], op0=OP.add, op1=OP.add)
                    ln.chunk(m)
                if debug and li == 0:
                    o = dbg_out("dbg_rT2", [128, DCH, TLOC])
                    nc.sync.dma_start(out=o[:, :, :], in_=rT[:])
                if li + 1 < n_layers:
                    ln.finish("g1", "be1", li + 1, ln_triv[2 * (li + 1)],
                              None)
                    prewarm(AF.Exp)
                else:
                    ln.finish("gf", "bef", 0, ln_triv[2 * n_layers], 0)

            # =================== logits ===================
            # gathered activations -> xtf [128, NC, DCH, TLOC]; each
            # feature-half loads as soon as its AllGather lands, and the
            # contraction consumes half A (d 0-3) before half B arrives.
            xtf = acts.tile([128, NC, DCH, TLOC], BF16, tag="bigact",
                            name="xtf")
            for k in range(2):
                agv = ag_out[k].ap().rearrange("(r p) c t -> p r c t", p=128)
                for q in range(4):
                    eng = nc.scalar if q % 2 == 0 else nc.gpsimd
                    eng.dma_start(
                        out=xtf[:, 2 * q:2 * q + 2,
                                HD2 * k:HD2 * (k + 1), :],
                        in_=agv[:, 2 * q:2 * q + 2, :, :])
            bout_sb = None
            if use_bout:
                bout_sb = konst.tile([VMC, VCH], F32)
                nc.sync.dma_start(out=bout_sb[:], in_=boutT[:, :])
            for q2 in range(16):
                wo_t = woutp.tile([128, 2, DCH, VMC], BF16, tag="wout",
                                  name=f"wo_t{q2}")
                nc.sync.dma_start(out=wo_t[:], in_=wout[q2])
                for mi in range(2):
                    m = 2 * q2 + mi
                    ot = otp.tile([VMC, T], BF16, tag="ot", name=f"ot{m}")
                    psa = psA.tile([VMC, 2, 512], F32, tag="a",
                                   name=f"plga{m}")
                    pps = [psa[:, 0, :], psa[:, 1, :]] + \
                          [psB.tile([VMC, 512], F32, tag="b",
                                    name=f"plgb{m}_{k}")[:] for k in range(2)]
                    for d in range(DCH):
                        for n in range(4):
                            nc.tensor.matmul(pps[n], wo_t[:, mi, d, :],
                                             xtf[:, 2 * n:2 * n + 2, d, :],
                                             start=(d == 0),
                                             stop=(d == DCH - 1))
                    for n in range(4):
                        if use_bout:
                            nc.vector.tensor_scalar(
                                ot[:, 512 * n:512 * (n + 1)], pps[n],
                                bout_sb[:, m:m + 1], None, op0=OP.add)
                        else:
                            nc.vector.tensor_copy(
                                ot[:, 512 * n:512 * (n + 1)], pps[n])
                    nc.scalar.dma_start(out=outT[VMC * m:VMC * (m + 1), :],
                                        in_=ot[:])

    nc.compile()
    return nc, sorted(dbg.keys())


def _prep_inputs(inputs):
    """Shard + transform full inputs -> list of 8 per-core in_maps."""
    f32 = lambda a: np.ascontiguousarray(np.asarray(a), dtype=np.float32)
    bf16 = lambda a: np.ascontiguousarray(np.asarray(a, dtype=np.float32)).astype(BF)

    x = np.asarray(inputs["x"]).astype(np.int64).reshape(T)
    tok = f32(inputs["tok_emb"])
    pos = f32(inputs["pos_emb"])
    Wq, Wk, Wv, Wo = (f32(inputs[k]) for k in ("Wq", "Wk", "Wv", "Wo"))
    W1, W2, Wout = f32(inputs["W1"]), f32(inputs["W2"]), f32(inputs["Wout"])
    bq, bk, bv = f32(inputs["bq"]), f32(inputs["bk"]), f32(inputs["bv"])
    bo, b1, b2 = f32(inputs["bo"]), f32(inputs["b1"]), f32(inputs["b2"])
    bout = f32(inputs["bout"])
    ln1_g, ln1_b = f32(inputs["ln1_g"]), f32(inputs["ln1_b"])
    ln2_g, ln2_b = f32(inputs["ln2_g"]), f32(inputs["ln2_b"])
    lnf_g, lnf_b = f32(inputs["lnf_g"]), f32(inputs["lnf_b"])

    def chunkT(vec_1d):  # [D] -> [128, DCH] (p, chunk)
        return np.ascontiguousarray(vec_1d.reshape(DCH, 128).T)

    # causal mask tiles: keep if f >= p + 128*r
    p_i = np.arange(128)[:, None]
    f_i = np.arange(TLOC)[None, :]
    cm = np.stack([(f_i >= p_i), (f_i >= p_i + 128)], 1).astype(BF)  # [128,2,256]
    idn = np.eye(128, dtype=BF)

    ln_triv = []
    for g, b in ((ln1_g, ln1_b), (ln2_g, ln2_b)):
        for i in range(L):
            ln_triv.append(bool(np.all(g[i] == 1.0) and np.all(b[i] == 0.0)))
    # interleave per layer: [ln1_0, ln2_0, ln1_1, ln2_1, ...]
    ln_triv = [ln_triv[i] if s == 0 else ln_triv[L + i]
               for i in range(L) for s in range(2)] + \
              [bool(np.all(lnf_g == 1.0) and np.all(lnf_b == 0.0))]
    use_bout = bool(np.any(bout))

    # [L, D, Do] -> [L, Do/256, 128(in-sub), 2(pair), DCH(in-chunk), 128]
    def qlay(w, do):
        a = w.reshape(L, DCH, 128, do // 128, 128).transpose(0, 3, 2, 1, 4)
        a = a.reshape(L, do // 256, 2, 128, DCH, 128).transpose(0, 1, 3, 2, 4, 5)
        return np.ascontiguousarray(a)

    in_maps = []
    for c in range(NC):
        xl = x[TLOC * c:TLOC * (c + 1)]
        embT_ = np.ascontiguousarray(tok[xl].T).reshape(DCH, 128, TLOC)
        embT_ = np.ascontiguousarray(embT_.transpose(1, 0, 2))  # [128, DCH, 256]
        p0 = (c % 4) * TLOC
        posT_ = np.ascontiguousarray(pos[p0:p0 + TLOC].T).reshape(DCH, 128, TLOC)
        posT_ = np.ascontiguousarray(posT_.transpose(1, 0, 2))
        lay = lambda w: np.ascontiguousarray(
            w.reshape(L, DCH, 128, w.shape[-1]).transpose(0, 2, 1, 3))
        w2h = np.ascontiguousarray(
            W2.reshape(L, 32, 128, DCH, 128).transpose(0, 3, 2, 1, 4))
        wouth = Wout[:, VS * c:VS * (c + 1)].reshape(DCH, 128, VCH, VMC) \
            .transpose(2, 1, 0, 3)  # [VCH, 128, DCH, VMC]
        wouth = wouth.reshape(16, 2, 128, DCH, VMC).transpose(0, 2, 1, 3, 4)
        wouth = np.ascontiguousarray(wouth)  # [16, 128, 2, DCH, VMC]
        m = dict(
            embT=embT_, posT=posT_,
            wq=bf16(qlay(Wq, D)), wk=bf16(qlay(Wk, D)), wv=bf16(qlay(Wv, D)),
            wo=bf16(lay(Wo.reshape(L, NC, 2, 64, D).transpose(0, 2, 1, 3, 4)
                        .reshape(L, D, D))),
            w1=bf16(qlay(W1, F)),
            w2=bf16(w2h), wout=bf16(wouth),
            bqkv=np.ascontiguousarray(
                np.stack([np.stack([chunkT(bb[i]) for i in range(L)], -1)
                          for bb in (bq, bk, bv)], 2)),  # [128,DCH,3,L]
            boT=np.ascontiguousarray(
                np.stack([chunkT(bo[i]) for i in range(L)], -1)),
            b1T=np.ascontiguousarray(np.stack(
                [np.ascontiguousarray(b1[i].reshape(32, 128).T)
                 for i in range(L)], -1)),
            b2T=np.ascontiguousarray(
                np.stack([chunkT(b2[i]) for i in range(L)], -1)),
            boutT=np.ascontiguousarray(
                bout[VS * c:VS * (c + 1)].reshape(VCH, VMC).T),
            g1T=np.ascontiguousarray(
                np.stack([chunkT(ln1_g[i]) for i in range(L)], -1)),
            be1T=np.ascontiguousarray(
                np.stack([chunkT(ln1_b[i]) for i in range(L)], -1)),
            g2T=np.ascontiguousarray(
                np.stack([chunkT(ln2_g[i]) for i in range(L)], -1)),
            be2T=np.ascontiguousarray(
                np.stack([chunkT(ln2_b[i]) for i in range(L)], -1)),
            gfT=chunkT(lnf_g)[:, :, None].copy(),
            befT=chunkT(lnf_b)[:, :, None].copy(),
            cmask=cm, ident=idn,
        )
        in_maps.append(m)
    return in_maps, use_bout, ln_triv


_CACHE = {}


def _get_nc(n_layers, use_bout, ln_triv, debug=False):
    key = (n_layers, use_bout, tuple(ln_triv), debug)
    if key not in _CACHE:
        _CACHE[key] = _build(n_layers, use_bout, ln_triv, debug)
    return _CACHE[key]


def run(inputs, n_layers=L, debug=False, trace=False):
    from concourse import bass_utils
    in_maps, use_bout, ln_triv = _prep_inputs(inputs)
    nc, dbg_names = _get_nc(n_layers, use_bout, ln_triv, debug)
    res = bass_utils.run_bass_kernel_spmd(
        nc, in_maps, core_ids=list(range(NC)), trace=trace)
    return res, dbg_names


def kernel(**inputs):
    res, _ = run(inputs)
    outs = [res.results[c]["outT"] for c in range(NC)]          # [4000, 2048] bf16
    logits = np.concatenate([np.asarray(o, np.float32).T for o in outs],
                            axis=1)                             # [2048, 32000]
    return np.ascontiguousarray(logits.reshape(B, S, V))
